# revision 1
# baseline (speedup 1.0000x reference)
"""Adaptive per-pixel LoG 9x9 convolution on 8 TRN2 NeuronCores.

out[b,c,y,x] = sum_{dy,dx in [-4,4]} xpad[b,c,y+dy,x+dx] * K(dx^2+dy^2; p)
K depends on the offset only through r2 = dx^2+dy^2 (15 distinct values)
-> exact rank-15 decomposition  out = sum_v Gp_v * S_v  where S_v are
fixed ring-sum convolutions (shared shifted adds) and Gp_v are the
host-computed per-pixel weight planes base*(1-t)exp(-t), t = r2*inv2s2
(plane 14 = base alone, weighting the center pixel).

Sharding: 8 cores = 4 batches x 2 row-halves. Partition p = 16x16 output
tile + 4px halo (24x24 window, 3 channels); all taps are free-dim AP
offsets; host bakes the window layout so DMAs are contiguous.

Engine choreography (all weight math lives on the host; GpSimd compute
serializes against the DVE so the Pool engine only issues DMAs):
- xp ships as two full-128-partition flat halves on the two HWDGE
  queues (partial-partition DMAs hit a 2-of-16 SDMA-engine pathology;
  per-queue DMAs complete serially at ~1.5-3us each, so each queue gets
  exactly one DMA per dependency tier); the weight planes Gp (960KB)
  ride third, needed only late.
- xp1 (1-col-shifted copy, keeps the odd column taps 4B-aligned for DVE
  2x mode) ships second on both queues; ACT copies the center-column
  class into U[4].
- DVE: column-class sums U, one merged signed-stride op for all 20
  row-pair sums D, ring assembly, products vs Gp (zero-stride channel
  broadcast), 5-op tree reduce; single bf16 output DMA.
"""

import math

import numpy as np

B, C, H, W = 4, 3, 256, 256
PAD = 4
SIGMA_MIN, SIGMA_MAX = 0.5, 10.0
N_CORES = 8

S_ROWS = 16
S_COLS = 16
N_STRIPS = 8
N_BLOCKS = 16
IN_R = 24
IN_C = 24
IN_C1 = 22

XP_FLAT = C * IN_R * IN_C      # 1728
XP1_FLAT = C * IN_R * IN_C1    # 1584

R2_VALUES = sorted({dx * dx + dy * dy for dx in range(-4, 5) for dy in range(-4, 5)})
assert len(R2_VALUES) == 15
NV = 15
# ring order: Gp planes 0..9 pair with S[1..10]; 10..13 with the D diagonals
V_ORD = [1, 4, 9, 16, 5, 10, 17, 13, 20, 25, 2, 8, 18, 32]


def _build_program(nc, bass, mybir):
    bf16 = mybir.dt.bfloat16
    Alu = mybir.AluOpType
    Act = mybir.ActivationFunctionType

    xp_d = nc.declare_dram_parameter("xp", [128, XP_FLAT], bf16, isOutput=False)
    xp1_d = nc.declare_dram_parameter("xp1", [128, XP1_FLAT], bf16, isOutput=False)
    gp_d = nc.declare_dram_parameter("gp", [128, NV, S_ROWS * S_COLS], bf16, isOutput=False)
    out_d = nc.declare_dram_parameter("out", [128, C, S_ROWS, S_COLS], bf16, isOutput=True)

    XA = XP_FLAT // 2
    X1A = XP1_FLAT // 2

    xa_sem = nc.alloc_semaphore("xa_sem")
    xb_sem = nc.alloc_semaphore("xb_sem")
    x1a_sem = nc.alloc_semaphore("x1a_sem")
    x1b_sem = nc.alloc_semaphore("x1b_sem")
    ga_sem = nc.alloc_semaphore("ga_sem")
    gb_sem = nc.alloc_semaphore("gb_sem")
    act_sem = nc.alloc_semaphore("act_sem")
    dve_sem = nc.alloc_semaphore("dve_sem")
    od_sem = nc.alloc_semaphore("od_sem")
    xp = nc.alloc_sbuf_tensor("s_xp", [128, C, IN_R, IN_C], bf16)
    xp1 = nc.alloc_sbuf_tensor("s_xp1", [128, C, IN_R, IN_C1], bf16)
    Gp = nc.alloc_sbuf_tensor("Gp", [128, NV, S_ROWS * S_COLS], bf16)
    U = nc.alloc_sbuf_tensor("U", [128, 5, C, IN_R, S_COLS], bf16)
    D = nc.alloc_sbuf_tensor("D", [128, 4, 5, C, S_ROWS, S_COLS], bf16)
    S = nc.alloc_sbuf_tensor("S", [128, 11, C, S_ROWS, S_COLS], bf16)
    P = nc.alloc_sbuf_tensor("P", [128, NV, C, S_ROWS, S_COLS], bf16)
    O = nc.alloc_sbuf_tensor("O", [128, C, S_ROWS, S_COLS], bf16)
    scratch = nc.alloc_sbuf_tensor("scratch", [128, 2], bf16)

    with nc.Block(no_gpsimd_drain=True) as block:
        def flat(t, lo, hi):
            # flat per-partition [lo:hi) element view of an SBUF tensor
            a = t[:]
            return bass.AP(t, lo, [list(a.ap[0]), [1, hi - lo]])

        @block.sync
        def _(sync):
            sync.dma_start(out=flat(xp, 0, XA), in_=xp_d[:, 0:XA]).then_inc(xa_sem, 16)
            sync.dma_start(out=flat(xp1, 0, X1A), in_=xp1_d[:, 0:X1A]).then_inc(
                x1a_sem, 16
            )
            # Gp is only needed ~14us later; gate it behind xp1 so the
            # 960KB of weight traffic never contends with any core's
            # critical xp/xp1 window on HBM
            sync.wait_ge(x1a_sem, 16)
            sync.dma_start(out=Gp[:, 0:8], in_=gp_d[:, 0:8]).then_inc(ga_sem, 16)
            sync.wait_ge(dve_sem, 1)
            sync.dma_start(out=out_d[:], in_=O[:]).then_inc(od_sem, 16)
            sync.wait_ge(od_sem, 16)

        @block.gpsimd
        def _(gpsimd):
            gpsimd.wait_ge(od_sem, 16)

        @block.scalar
        def _(scalar):
            scalar.dma_start(out=flat(xp, XA, XP_FLAT), in_=xp_d[:, XA:]).then_inc(
                xb_sem, 16
            )
            scalar.dma_start(
                out=flat(xp1, X1A, XP1_FLAT), in_=xp1_d[:, X1A:]
            ).then_inc(x1b_sem, 16)
            # dummy op hoists the lazy 1.3us ACT_TABLE_LOAD off the U4 path
            zero_ap = nc.const_aps.aps[(mybir.dt.float32, 0.0)]
            scalar.activation(scratch[:, 0:1], zero_ap, Act.Copy)
            # center-column class U[4] for the merged row-pair op (ACT is idle)
            scalar.wait_ge(xa_sem, 16)
            scalar.wait_ge(xb_sem, 16)
            scalar.activation(
                U[:, 4], xp[:, :, :, PAD : PAD + S_COLS], Act.Copy
            ).then_inc(act_sem, 1)
            scalar.wait_ge(x1b_sem, 16)
            scalar.dma_start(out=Gp[:, 8:NV], in_=gp_d[:, 8:NV]).then_inc(gb_sem, 16)
            scalar.wait_ge(od_sem, 16)

        @block.vector
        def _(vector):
            pU = list(U[:].ap[0])
            # stage 1: column-class sums U1/U3 from xp
            vector.wait_ge(xa_sem, 16)
            vector.wait_ge(xb_sem, 16)
            vector.tensor_tensor(
                U[:, 1], xp[:, :, :, 2 : 2 + S_COLS], xp[:, :, :, 6 : 6 + S_COLS], Alu.add
            )
            vector.tensor_tensor(
                U[:, 3], xp[:, :, :, 0:S_COLS], xp[:, :, :, 8 : 8 + S_COLS], Alu.add
            )
            # stage 1b: U0/U2 from the shifted copy xp1
            vector.wait_ge(x1a_sem, 16)
            vector.wait_ge(x1b_sem, 16)
            vector.tensor_tensor(
                U[:, 0], xp1[:, :, :, 2 : 2 + S_COLS], xp1[:, :, :, 4 : 4 + S_COLS], Alu.add
            )
            vector.tensor_tensor(
                U[:, 2], xp1[:, :, :, 0:S_COLS], xp1[:, :, :, 6 : 6 + S_COLS], Alu.add
            )

            # stage 2a: symmetric row-pair sums, all k and 5 col classes
            # (class 4 = center cols, copied into U[4] by the ACT engine)
            vector.wait_ge(act_sem, 1)
            vector.tensor_tensor(
                D[:],
                bass.AP(U, 48, [pU, [-16, 4], [1152, 5], [384, 3], [16, S_ROWS], [1, S_COLS]]),
                bass.AP(U, 80, [pU, [16, 4], [1152, 5], [384, 3], [16, S_ROWS], [1, S_COLS]]),
                Alu.add,
            )

            def dview_outer(k, a, n, stride):
                # n D[k, a + i] or D[k + i, a] planes stepping by `stride`
                src2 = D[:, k, a]
                return bass.AP(
                    D,
                    src2.offset,
                    [list(src2.ap[0]), [stride, n]] + [list(x) for x in src2.ap[1:]],
                )

            # stage 2b: ring assembly
            # centers: S[1..4] = U_a[dy=0] + D[k=a, center]  (v = 1,4,9,16)
            vector.tensor_tensor(
                S[:, 1:5],
                bass.AP(U, PAD * S_COLS, [pU, [1152, 4], [384, 3], [16, S_ROWS], [1, S_COLS]]),
                dview_outer(0, 4, 4, 3840),
                Alu.add,
            )
            # mixed pairs: S[5..7] = D[1,{2,3,4}] + D[{2,3,4},1]  (v = 5,10,17)
            vector.tensor_tensor(
                S[:, 5:8], dview_outer(0, 1, 3, 768), dview_outer(1, 0, 3, 3840), Alu.add
            )
            # S[8..9] = D[2,{3,4}] + D[{3,4},2]  (v = 13,20)
            vector.tensor_tensor(
                S[:, 8:10], dview_outer(1, 2, 2, 768), dview_outer(2, 1, 2, 3840), Alu.add
            )
            # S[10] = D[3,4] + D[4,3]  (v = 25)
            vector.tensor_tensor(S[:, 10], D[:, 2, 3], D[:, 3, 2], Alu.add)

            def gbc(i, n):
                # Gp planes [i:i+n) broadcast over the channel dim
                a = Gp[:]
                return bass.AP(
                    Gp,
                    i * 256,
                    [list(a.ap[0]), [256, n], [0, C], [S_COLS, S_ROWS], [1, S_COLS]],
                )

            # products: P[0..9] = S[1..10]*g, P[10..13] = D[j,j]*g,
            # P[14] = center pixel * base
            vector.wait_ge(ga_sem, 16)
            vector.tensor_tensor(P[:, 0:8], S[:, 1:9], gbc(0, 8), Alu.mult)
            vector.wait_ge(gb_sem, 16)
            vector.tensor_tensor(P[:, 8:10], S[:, 9:11], gbc(8, 2), Alu.mult)
            vector.tensor_tensor(P[:, 10:14], dview_outer(0, 0, 4, 4608), gbc(10, 4), Alu.mult)
            vector.tensor_tensor(
                P[:, 14],
                xp[:, :, PAD : PAD + S_ROWS, PAD : PAD + S_COLS],
                gbc(14, 1),
                Alu.mult,
            )

            # tree-reduce the 15 products (5 ops), last one writes O
            vector.tensor_tensor(P[:, 0:7], P[:, 0:7], P[:, 7:14], Alu.add)
            vector.tensor_tensor(P[:, 0:3], P[:, 0:3], P[:, 3:6], Alu.add)
            # P[0]+=P[2], P[1]+=P[6] in one op
            p2 = P[:, 2]
            vector.tensor_tensor(
                P[:, 0:2],
                P[:, 0:2],
                bass.AP(
                    P,
                    p2.offset,
                    [list(p2.ap[0]), [4 * 768, 2]] + [list(x) for x in p2.ap[1:]],
                ),
                Alu.add,
            )
            vector.tensor_tensor(P[:, 0], P[:, 0], P[:, 1], Alu.add)
            vector.tensor_tensor(O[:], P[:, 0], P[:, 14], Alu.add).then_inc(dve_sem, 1)

    return nc


_PROGRAM_CACHE = {}


def _get_program():
    if "nc" not in _PROGRAM_CACHE:
        import sys

        if "/opt/trn_rl_repo" not in sys.path:
            sys.path.insert(0, "/opt/trn_rl_repo")
        from concourse import bass, mybir

        nc = bass.Bass()
        _PROGRAM_CACHE["nc"] = _build_program(nc, bass, mybir)
    return _PROGRAM_CACHE["nc"]


def _host_prep(x, foa_xy):
    import ml_dtypes

    bf = ml_dtypes.bfloat16
    xpad = np.pad(x, ((0, 0), (0, 0), (PAD, PAD), (PAD, PAD)), mode="reflect")
    xpad_bf = xpad.astype(bf)
    diag = math.sqrt(H * H + W * W)
    in_maps = []
    for core in range(N_CORES):
        b, half = divmod(core, 2)
        y0 = half * 128
        xph = xpad_bf[b, :, y0 : y0 + 136, :]
        sw = np.lib.stride_tricks.sliding_window_view(xph, (C, IN_R, IN_C))
        XP = np.ascontiguousarray(
            sw[0, ::S_ROWS, ::S_COLS].reshape(128, XP_FLAT)
        )
        sw1 = np.lib.stride_tricks.sliding_window_view(xph, (C, IN_R, IN_C1))
        XP1 = np.ascontiguousarray(
            sw1[0, ::S_ROWS, 1::S_COLS][:, :N_BLOCKS].reshape(128, XP1_FLAT)
        )

        yy, xx = np.meshgrid(
            np.arange(y0, y0 + 128, dtype=np.float64),
            np.arange(W, dtype=np.float64),
            indexing="ij",
        )
        fx, fy = float(foa_xy[b, 0]), float(foa_xy[b, 1])
        dist = np.sqrt((xx - fx) ** 2 + (yy - fy) ** 2)
        dn = dist / diag
        sigma = (1.0 - dn) * SIGMA_MIN + dn * SIGMA_MAX
        inv2s2 = 1.0 / (2.0 * sigma * sigma)
        base = -dist * np.sqrt(sigma) / (math.pi * sigma**4)

        def tiles(a):
            t = a.reshape(N_STRIPS, S_ROWS, N_BLOCKS, S_COLS)
            return t.transpose(0, 2, 1, 3).reshape(128, S_ROWS * S_COLS)

        bt, it = tiles(base), tiles(inv2s2)
        GP = np.empty((128, NV, S_ROWS * S_COLS), dtype=bf)
        for i, v in enumerate(V_ORD):
            t = v * it
            GP[:, i] = (bt * (1.0 - t) * np.exp(-t)).astype(bf)
        GP[:, 14] = bt.astype(bf)

        in_maps.append({"xp": XP, "xp1": XP1, "gp": np.ascontiguousarray(GP)})
    return in_maps


def _gather(results):
    out = np.empty((B, C, H, W), dtype=np.float32)
    for core in range(N_CORES):
        b, half = divmod(core, 2)
        y0 = half * 128
        o = results[core]["out"].astype(np.float32)
        o = o.reshape(N_STRIPS, N_BLOCKS, C, S_ROWS, S_COLS)
        o = o.transpose(2, 0, 3, 1, 4).reshape(C, 128, W)
        out[b, :, y0 : y0 + 128, :] = o
    return out


def kernel(x, foa_xy, _trace=False, _tmpdir=None):
    import sys

    if "/opt/trn_rl_repo" not in sys.path:
        sys.path.insert(0, "/opt/trn_rl_repo")
    from concourse.bass_utils import run_bass_kernel_spmd

    nc = _get_program()
    in_maps = _host_prep(np.asarray(x), np.asarray(foa_xy))
    kw = {}
    if _trace:
        kw = dict(trace=True, trace_cores=[], tmpdir=_tmpdir)
    res = run_bass_kernel_spmd(nc, in_maps, list(range(N_CORES)), **kw)
    out = _gather(res.results)
    if _trace:
        return out, res
    return out



# revision 5
# speedup vs baseline: 1.1177x; 1.1177x over previous
"""Adaptive per-pixel LoG 9x9 convolution on 8 TRN2 NeuronCores.

out[b,c,y,x] = sum_{dy,dx in [-4,4]} xpad[b,c,y+dy,x+dx] * K(dx^2+dy^2; p)
K depends on the offset only through r2 = dx^2+dy^2 (15 distinct values)
-> exact rank-15 decomposition  out = sum_v Gp_v * S_v  where S_v are
fixed ring-sum convolutions and Gp_v are host-computed per-pixel weight
planes.

Row-partition layout: 8 cores = 4 batches x 2 row-halves; partition p =
image row p of the half (half1 is vertically flipped by the host so one
SPMD program serves all cores; reflect at the image edge is baked into
the stationary matrices, and the 4 bottom rows' taps that fall outside
the 128-row window arrive as a tiny host-computed bias plane).

Engine split (vs. the all-DVE tile-layout baseline):
- DVE: 2 merged column-class sum ops (U), per-unit products vs Gp,
  tree-reduce + bias add. ~14us instead of ~28us.
- PE: all row-band/ring accumulation = 25 banded-stationary matmuls per
  (channel, column-half) unit into PSUM (fp32), FD=128, double-buffered
  across 2x4 PSUM banks.
- ACT: evacuates each unit's 15 S-planes PSUM->SBUF bf16.
- Output DMAed per column-half to overlap the tail.
"""

import math

import numpy as np

B, C, H, W = 4, 3, 256, 256
PAD = 4
SIGMA_MIN, SIGMA_MAX = 0.5, 10.0
N_CORES = 8
DIAG = math.sqrt(H * H + W * W)

NV = 15
V_ORD = [1, 4, 9, 16, 5, 10, 17, 13, 20, 25, 2, 8, 18, 32]  # + center v=0 at 14

# plane -> [(d=|dy| class, acl)], acl: 0..3 = |dx| 1..4, 4 = center col
PLANE_TERMS = [
    [(0, 0), (1, 4)],   # v1
    [(0, 1), (2, 4)],   # v4
    [(0, 2), (3, 4)],   # v9
    [(0, 3), (4, 4)],   # v16
    [(1, 1), (2, 0)],   # v5
    [(1, 2), (3, 0)],   # v10
    [(1, 3), (4, 0)],   # v17
    [(2, 2), (3, 1)],   # v13
    [(2, 3), (4, 1)],   # v20
    [(3, 3), (4, 2)],   # v25
    [(1, 0)],           # v2
    [(2, 1)],           # v8
    [(3, 2)],           # v18
    [(4, 3)],           # v32
    [(0, 4)],           # v0 center
]
ACL_DX = [[-1, 1], [-2, 2], [-3, 3], [-4, 4], [0]]

# plane-major matmul emission order: each plane's accumulation group is
# consecutive (interleaved start/stop groups mis-accumulate on HW)
_EMIT = []
for _p, _terms in enumerate(PLANE_TERMS):
    for _i, (_td, _ta) in enumerate(sorted(_terms)):
        _EMIT.append((_p, _ta, _td, _i == 0, _i == len(_terms) - 1))
assert len(_EMIT) == 25

XP_FLAT = C * 264      # 792
XP1_FLAT = C * 262     # 786
G_FLAT = NV * 256      # 3840
BST_FLAT = 5 * 128     # 640
O_FLAT = 2 * C * 128   # 768


def _build_program(nc, bass, mybir):
    bf16 = mybir.dt.bfloat16
    f32 = mybir.dt.float32
    Alu = mybir.AluOpType
    Act = mybir.ActivationFunctionType

    xp_d = nc.declare_dram_parameter("xp", [128, XP_FLAT], bf16, isOutput=False)
    xp1_d = nc.declare_dram_parameter("xp1", [128, XP1_FLAT], bf16, isOutput=False)
    g_d = nc.declare_dram_parameter("g", [128, G_FLAT], bf16, isOutput=False)
    bst_d = nc.declare_dram_parameter("bst", [128, BST_FLAT], bf16, isOutput=False)
    bias_d = nc.declare_dram_parameter("bias", [128, O_FLAT], bf16, isOutput=False)
    out_d = nc.declare_dram_parameter("out", [128, O_FLAT], bf16, isOutput=True)

    xa_sem = nc.alloc_semaphore("xa_sem")
    xb_sem = nc.alloc_semaphore("xb_sem")
    x1a_sem = nc.alloc_semaphore("x1a_sem")
    x1b_sem = nc.alloc_semaphore("x1b_sem")
    bst_sem = nc.alloc_semaphore("bst_sem")
    ga_sem = nc.alloc_semaphore("ga_sem")
    gb_sem = nc.alloc_semaphore("gb_sem")
    bis_sem = nc.alloc_semaphore("bis_sem")
    u_sem = nc.alloc_semaphore("u_sem")
    pe_sem = nc.alloc_semaphore("pe_sem")
    act_sem = nc.alloc_semaphore("act_sem")
    dh0_sem = nc.alloc_semaphore("dh0_sem")
    dh1_sem = nc.alloc_semaphore("dh1_sem")
    od0_sem = nc.alloc_semaphore("od0_sem")
    od1_sem = nc.alloc_semaphore("od1_sem")

    xp = nc.alloc_sbuf_tensor("s_xp", [128, C, 264], bf16)
    xp1 = nc.alloc_sbuf_tensor("s_xp1", [128, C, 262], bf16)
    U = nc.alloc_sbuf_tensor("U", [128, 4, C, 256], bf16)
    Ssb = nc.alloc_sbuf_tensor("Ssb", [128, NV, C, 256], bf16)
    P = nc.alloc_sbuf_tensor("P", [128, NV, C, 256], bf16)
    G = nc.alloc_sbuf_tensor("G", [128, NV, 256], bf16)
    BST = nc.alloc_sbuf_tensor("BST", [128, 5, 128], bf16)
    BIAS = nc.alloc_sbuf_tensor("BIAS", [128, 2, C, 128], bf16)
    O = nc.alloc_sbuf_tensor("O", [128, 2, C, 128], bf16)
    scratch = nc.alloc_sbuf_tensor("scratch", [128, 2], bf16)

    ps = [
        nc.alloc_psum_tensor("ps0", [128, NV, 128], f32),
        nc.alloc_psum_tensor("ps1", [128, NV, 128], f32),
    ]

    XA = 396
    X1A = 392
    GA = 1920

    with nc.Block(no_gpsimd_drain=True) as block:
        def flat(t, lo, hi):
            a = t[:]
            return bass.AP(t, lo, [list(a.ap[0]), [1, hi - lo]])

        def pdims(t):
            return list(t[:].ap[0])

        @block.sync
        def _(sync):
            sync.dma_start(out=flat(xp, 0, XA), in_=xp_d[:, 0:XA]).then_inc(xa_sem, 16)
            sync.dma_start(out=flat(xp1, 0, X1A), in_=xp1_d[:, 0:X1A]).then_inc(
                x1a_sem, 16
            )
            sync.dma_start(out=flat(BST, 0, BST_FLAT), in_=bst_d[:]).then_inc(
                bst_sem, 16
            )
            sync.dma_start(out=flat(G, 0, GA), in_=g_d[:, 0:GA]).then_inc(ga_sem, 16)
            sync.wait_ge(dh0_sem, 1)
            sync.dma_start(out=out_d[:, 0:384], in_=flat(O, 0, 384)).then_inc(
                od0_sem, 16
            )
            sync.wait_ge(od0_sem, 16)
            sync.wait_ge(od1_sem, 16)

        @block.gpsimd
        def _(gpsimd):
            gpsimd.wait_ge(od0_sem, 16)
            gpsimd.wait_ge(od1_sem, 16)

        @block.scalar
        def _(scalar):
            scalar.dma_start(out=flat(xp, XA, XP_FLAT), in_=xp_d[:, XA:]).then_inc(
                xb_sem, 16
            )
            scalar.dma_start(
                out=flat(xp1, X1A, XP1_FLAT), in_=xp1_d[:, X1A:]
            ).then_inc(x1b_sem, 16)
            scalar.dma_start(out=flat(G, GA, G_FLAT), in_=g_d[:, GA:]).then_inc(
                gb_sem, 16
            )
            scalar.dma_start(out=flat(BIAS, 0, O_FLAT), in_=bias_d[:]).then_inc(
                bis_sem, 16
            )
            # dummy op hoists the lazy ACT_TABLE_LOAD off the critical path
            zero_ap = nc.const_aps.aps[(mybir.dt.float32, 0.0)]
            scalar.activation(scratch[:, 0:1], zero_ap, Act.Copy)
            for u in range(6):
                h, c = divmod(u, 3)
                hs = 128 * h
                scalar.wait_ge(pe_sem, u + 1)
                scalar.activation(
                    Ssb[:, :, c, hs:hs + 128], ps[u % 2][:], Act.Copy
                ).then_inc(act_sem, 1)
            scalar.wait_ge(dh1_sem, 1)
            scalar.dma_start(out=out_d[:, 384:768], in_=flat(O, 384, 768)).then_inc(
                od1_sem, 16
            )
            scalar.wait_ge(od1_sem, 16)

        @block.tensor
        def _(tensor):
            tensor.wait_ge(bst_sem, 16)
            tensor.wait_ge(u_sem, 1)
            for u in range(6):
                h, c = divmod(u, 3)
                hs = 128 * h
                if u >= 2:
                    tensor.wait_ge(act_sem, u - 1)
                for i, (plane, acl, d, st, sp) in enumerate(_EMIT):
                    if acl == 4:
                        rhs = xp[:, c, 4 + hs:4 + hs + 128]
                    else:
                        rhs = U[:, acl, c, hs:hs + 128]
                    mm = tensor.matmul(
                        ps[u % 2][:, plane],
                        BST[:, d],
                        rhs,
                        start=st,
                        stop=sp,
                        skip_group_check=True,
                    )
                    if i == len(_EMIT) - 1:
                        mm.then_inc(pe_sem, 1)

        @block.vector
        def _(vector):
            # U column-class sums, classes {0,2} from xp1 and {1,3} from xp,
            # merged pairwise via signed outer strides
            vector.wait_ge(x1a_sem, 16)
            vector.wait_ge(x1b_sem, 16)
            vector.tensor_tensor(
                bass.AP(U, 0, [pdims(U), [1536, 2], [256, C], [1, 256]]),
                bass.AP(xp1, 2, [pdims(xp1), [-2, 2], [262, C], [1, 256]]),
                bass.AP(xp1, 4, [pdims(xp1), [2, 2], [262, C], [1, 256]]),
                Alu.add,
            )
            vector.wait_ge(xa_sem, 16)
            vector.wait_ge(xb_sem, 16)
            vector.tensor_tensor(
                bass.AP(U, 768, [pdims(U), [1536, 2], [256, C], [1, 256]]),
                bass.AP(xp, 2, [pdims(xp), [-2, 2], [264, C], [1, 256]]),
                bass.AP(xp, 6, [pdims(xp), [2, 2], [264, C], [1, 256]]),
                Alu.add,
            ).then_inc(u_sem, 1)

            def tree(h):
                hs = 128 * h
                vector.tensor_tensor(
                    P[:, 0:7, :, hs:hs + 128],
                    P[:, 0:7, :, hs:hs + 128],
                    P[:, 7:14, :, hs:hs + 128],
                    Alu.add,
                )
                vector.tensor_tensor(
                    P[:, 0:3, :, hs:hs + 128],
                    P[:, 0:3, :, hs:hs + 128],
                    P[:, 3:6, :, hs:hs + 128],
                    Alu.add,
                )
                p2 = P[:, 2, :, hs:hs + 128]
                vector.tensor_tensor(
                    P[:, 0:2, :, hs:hs + 128],
                    P[:, 0:2, :, hs:hs + 128],
                    bass.AP(
                        P,
                        p2.offset,
                        [list(p2.ap[0]), [4 * C * 256, 2]]
                        + [list(x) for x in p2.ap[1:]],
                    ),
                    Alu.add,
                )
                vector.tensor_tensor(
                    P[:, 0, :, hs:hs + 128],
                    P[:, 0, :, hs:hs + 128],
                    P[:, 1, :, hs:hs + 128],
                    Alu.add,
                )
                vector.tensor_tensor(
                    O[:, h], P[:, 0, :, hs:hs + 128], P[:, 14, :, hs:hs + 128],
                    Alu.add,
                )
                return vector.tensor_tensor(O[:, h], O[:, h], BIAS[:, h], Alu.add)

            for u in range(6):
                h, c = divmod(u, 3)
                hs = 128 * h
                vector.wait_ge(act_sem, u + 1)
                if u == 0:
                    vector.wait_ge(ga_sem, 16)
                    vector.wait_ge(gb_sem, 16)
                vector.tensor_tensor(
                    P[:, :, c, hs:hs + 128],
                    Ssb[:, :, c, hs:hs + 128],
                    G[:, :, hs:hs + 128],
                    Alu.mult,
                )
                if u == 2:
                    vector.wait_ge(bis_sem, 16)
                    tree(0).then_inc(dh0_sem, 1)
                if u == 5:
                    tree(1).then_inc(dh1_sem, 1)

    return nc


_PROGRAM_CACHE = {}


def _get_program():
    if "nc" not in _PROGRAM_CACHE:
        import sys

        if "/opt/trn_rl_repo" not in sys.path:
            sys.path.insert(0, "/opt/trn_rl_repo")
        from concourse import bass, mybir

        nc = bass.Bass()
        _PROGRAM_CACHE["nc"] = _build_program(nc, bass, mybir)
    return _PROGRAM_CACHE["nc"]


def _build_bst():
    bst = np.zeros((5, 128, 128), np.float32)  # [d, in row i, out row r]
    for d in range(5):
        for r in range(128):
            for s in ({d, -d} if d else {0}):
                i = r + s
                if i < 0:
                    i = -i  # top reflect
                if i <= 127:
                    bst[d, i, r] += 1.0
    return bst


def _host_prep(x, foa_xy):
    import ml_dtypes

    bf = ml_dtypes.bfloat16
    x = np.asarray(x)
    bst = _build_bst().transpose(1, 0, 2)  # [i, d, r]
    bst_flat = np.ascontiguousarray(bst.reshape(128, BST_FLAT).astype(bf))
    in_maps = []
    for core in range(N_CORES):
        b, half = divmod(core, 2)
        xb = x[b] if half == 0 else x[b][:, ::-1, :]
        xw = xb[:, 0:132, :]
        xpad = np.pad(xw, ((0, 0), (0, 0), (PAD, PAD)), mode="reflect")  # [3,132,264]
        xp = np.ascontiguousarray(xpad[:, 0:128, :].transpose(1, 0, 2)).astype(bf)
        xp1 = np.ascontiguousarray(xpad[:, 0:128, 1:263].transpose(1, 0, 2)).astype(bf)

        rp = np.arange(128)
        yy_img = rp if half == 0 else 255 - rp
        yy, xx = np.meshgrid(
            yy_img.astype(np.float64), np.arange(W, dtype=np.float64), indexing="ij"
        )
        fx, fy = float(foa_xy[b, 0]), float(foa_xy[b, 1])
        dist = np.sqrt((xx - fx) ** 2 + (yy - fy) ** 2)
        dn = dist / DIAG
        sigma = (1.0 - dn) * SIGMA_MIN + dn * SIGMA_MAX
        inv2s2 = 1.0 / (2.0 * sigma * sigma)
        base = -dist * np.sqrt(sigma) / (math.pi * sigma ** 4)
        Gf = np.empty((128, NV, 256), np.float32)
        for i, v in enumerate(V_ORD):
            t = v * inv2s2
            Gf[:, i] = base * (1.0 - t) * np.exp(-t)
        Gf[:, 14] = base

        # bias for out rows 124..127: taps at rows 128..131 (outside window)
        rows = xpad[:, 128:132, :].astype(np.float32)  # [3, 4, 264]
        cs = np.zeros((5, 4, C, 256), np.float32)
        for a in range(5):
            for dx in ACL_DX[a]:
                cs[a] += rows[:, :, 4 + dx:4 + dx + 256].transpose(1, 0, 2)
        bias = np.zeros((128, C, 256), np.float32)
        for plane, terms in enumerate(PLANE_TERMS):
            for (d, acl) in terms:
                if d == 0:
                    continue
                for r in range(124, 128):
                    i = r + d
                    if i >= 128:
                        bias[r] += Gf[r, plane][None, :] * cs[acl, i - 128]
        # [128, 2, C, 128] half-major
        bias_t = np.ascontiguousarray(
            bias.reshape(128, C, 2, 128).transpose(0, 2, 1, 3)
        ).astype(bf)

        in_maps.append(
            {
                "xp": np.ascontiguousarray(xp.reshape(128, XP_FLAT)),
                "xp1": np.ascontiguousarray(xp1.reshape(128, XP1_FLAT)),
                "g": np.ascontiguousarray(Gf.astype(bf).reshape(128, G_FLAT)),
                "bst": bst_flat,
                "bias": bias_t.reshape(128, O_FLAT),
            }
        )
    return in_maps


def _gather(results):
    out = np.empty((B, C, H, W), dtype=np.float32)
    for core in range(N_CORES):
        b, half = divmod(core, 2)
        o = results[core]["out"].astype(np.float32).reshape(128, 2, C, 128)
        o = o.transpose(2, 0, 1, 3).reshape(C, 128, 256)
        if half:
            o = o[:, ::-1, :]
        out[b, :, half * 128:half * 128 + 128, :] = o
    return out


def kernel(x, foa_xy, _trace=False, _tmpdir=None):
    import sys

    if "/opt/trn_rl_repo" not in sys.path:
        sys.path.insert(0, "/opt/trn_rl_repo")
    from concourse.bass_utils import run_bass_kernel_spmd

    nc = _get_program()
    in_maps = _host_prep(np.asarray(x), np.asarray(foa_xy))
    kw = {}
    if _trace:
        kw = dict(trace=True, trace_cores=[], tmpdir=_tmpdir)
    res = run_bass_kernel_spmd(nc, in_maps, list(range(N_CORES)), **kw)
    out = _gather(res.results)
    if _trace:
        return out, res
    return out


# revision 6
# speedup vs baseline: 1.1211x; 1.0030x over previous
"""Adaptive per-pixel LoG 9x9 convolution on 8 TRN2 NeuronCores.

out[b,c,y,x] = sum_{dy,dx in [-4,4]} xpad[b,c,y+dy,x+dx] * K(dx^2+dy^2; p)
K depends on the offset only through r2 = dx^2+dy^2 (15 distinct values)
-> exact rank-15 decomposition  out = sum_v Gp_v * S_v  where S_v are
fixed ring-sum convolutions and Gp_v are host-computed per-pixel weight
planes.

Row-partition layout: 8 cores = 4 batches x 2 row-halves; partition p =
image row p of the half (half1 is vertically flipped by the host so one
SPMD program serves all cores; reflect at the image edge is baked into
the stationary matrices, and the 4 bottom rows' taps that fall outside
the 128-row window arrive as a tiny host-computed bias plane).

Engine split (vs. the all-DVE tile-layout baseline):
- DVE: 6 per-channel column-class sum ops (U), per-unit products vs Gp,
  tree-reduce + bias add. ~14us instead of ~28us.
- PE: all row-band/ring accumulation = 25 banded-stationary matmuls per
  (channel, column-half) unit into PSUM (fp32), FD=128, plane-major
  accumulation groups (interleaved groups mis-accumulate), PSUM
  double-buffered 2x4 banks.
- ACT: evacuates each unit's 15 S-planes PSUM->SBUF bf16 (unit 0 split
  in two so the DVE product stream starts earlier).
- Output DMAed per column-half; the last half rides both queues.
"""

import math

import numpy as np

B, C, H, W = 4, 3, 256, 256
PAD = 4
SIGMA_MIN, SIGMA_MAX = 0.5, 10.0
N_CORES = 8
DIAG = math.sqrt(H * H + W * W)

NV = 15
V_ORD = [1, 4, 9, 16, 5, 10, 17, 13, 20, 25, 2, 8, 18, 32]  # + center v=0 at 14

# plane -> [(d=|dy| class, acl)], acl: 0..3 = |dx| 1..4, 4 = center col
PLANE_TERMS = [
    [(0, 0), (1, 4)],   # v1
    [(0, 1), (2, 4)],   # v4
    [(0, 2), (3, 4)],   # v9
    [(0, 3), (4, 4)],   # v16
    [(1, 1), (2, 0)],   # v5
    [(1, 2), (3, 0)],   # v10
    [(1, 3), (4, 0)],   # v17
    [(2, 2), (3, 1)],   # v13
    [(2, 3), (4, 1)],   # v20
    [(3, 3), (4, 2)],   # v25
    [(1, 0)],           # v2
    [(2, 1)],           # v8
    [(3, 2)],           # v18
    [(4, 3)],           # v32
    [(0, 4)],           # v0 center
]
ACL_DX = [[-1, 1], [-2, 2], [-3, 3], [-4, 4], [0]]

# plane-major matmul emission order: each plane's accumulation group is
# consecutive (interleaved start/stop groups mis-accumulate on HW)
_EMIT = []
for _p, _terms in enumerate(PLANE_TERMS):
    for _i, (_td, _ta) in enumerate(sorted(_terms)):
        _EMIT.append((_p, _ta, _td, _i == 0, _i == len(_terms) - 1))
assert len(_EMIT) == 25
# index of the matmul that completes plane 7 (planes 0..7 evacuate first
# for unit 0)
_MID = sum(len(PLANE_TERMS[p]) for p in range(8)) - 1  # = 15

XP_N = C * 264         # 792
XP1_N = C * 262        # 786
XALL_FLAT = XP_N + XP1_N
G_FLAT = NV * 256      # 3840
BST_FLAT = 5 * 128     # 640
O_FLAT = 2 * C * 128   # 768


def _build_program(nc, bass, mybir):
    bf16 = mybir.dt.bfloat16
    f32 = mybir.dt.float32
    Alu = mybir.AluOpType
    Act = mybir.ActivationFunctionType

    xall_d = nc.declare_dram_parameter("xall", [128, XALL_FLAT], bf16, isOutput=False)
    g_d = nc.declare_dram_parameter("g", [128, G_FLAT], bf16, isOutput=False)
    bst_d = nc.declare_dram_parameter("bst", [128, BST_FLAT], bf16, isOutput=False)
    bias_d = nc.declare_dram_parameter("bias", [128, O_FLAT], bf16, isOutput=False)
    out_d = nc.declare_dram_parameter("out", [128, O_FLAT], bf16, isOutput=True)

    xa_sem = nc.alloc_semaphore("xa_sem")      # xp half (sync queue)
    x1a_sem = nc.alloc_semaphore("x1a_sem")    # xp1 half (scalar queue)
    bst_sem = nc.alloc_semaphore("bst_sem")
    ga_sem = nc.alloc_semaphore("ga_sem")
    gb_sem = nc.alloc_semaphore("gb_sem")
    bis_sem = nc.alloc_semaphore("bis_sem")
    u_sem = nc.alloc_semaphore("u_sem")
    pe_mid_sem = nc.alloc_semaphore("pe_mid_sem")
    pe_sem = nc.alloc_semaphore("pe_sem")
    act_sem = nc.alloc_semaphore("act_sem")
    dh0_sem = nc.alloc_semaphore("dh0_sem")
    dh1_sem = nc.alloc_semaphore("dh1_sem")
    od0_sem = nc.alloc_semaphore("od0_sem")
    od1_sem = nc.alloc_semaphore("od1_sem")

    xall = nc.alloc_sbuf_tensor("s_xall", [128, XALL_FLAT], bf16)
    U = nc.alloc_sbuf_tensor("U", [128, 4, C, 256], bf16)
    Ssb = nc.alloc_sbuf_tensor("Ssb", [128, NV, C, 256], bf16)
    P = nc.alloc_sbuf_tensor("P", [128, NV, C, 256], bf16)
    G = nc.alloc_sbuf_tensor("G", [128, NV, 256], bf16)
    BST = nc.alloc_sbuf_tensor("BST", [128, 5, 128], bf16)
    BIAS = nc.alloc_sbuf_tensor("BIAS", [128, 2, C, 128], bf16)
    O = nc.alloc_sbuf_tensor("O", [128, 2, C, 128], bf16)
    scratch = nc.alloc_sbuf_tensor("scratch", [128, 2], bf16)

    ps = [
        nc.alloc_psum_tensor("ps0", [128, NV, 128], f32),
        nc.alloc_psum_tensor("ps1", [128, NV, 128], f32),
    ]

    GA = 1920

    with nc.Block(no_gpsimd_drain=True) as block:
        def flat(t, lo, hi):
            a = t[:]
            return bass.AP(t, lo, [list(a.ap[0]), [1, hi - lo]])

        pd_xall = None  # filled below

        @block.sync
        def _(sync):
            sync.dma_start(out=flat(xall, 0, XP_N), in_=xall_d[:, 0:XP_N]).then_inc(
                xa_sem, 16
            )
            sync.dma_start(out=flat(BST, 0, BST_FLAT), in_=bst_d[:]).then_inc(
                bst_sem, 16
            )
            sync.dma_start(out=flat(G, 0, GA), in_=g_d[:, 0:GA]).then_inc(ga_sem, 16)
            sync.wait_ge(dh0_sem, 1)
            sync.dma_start(out=out_d[:, 0:384], in_=flat(O, 0, 384)).then_inc(
                od0_sem, 16
            )
            sync.wait_ge(dh1_sem, 1)
            sync.dma_start(out=out_d[:, 384:576], in_=flat(O, 384, 576)).then_inc(
                od1_sem, 16
            )
            sync.wait_ge(od0_sem, 16)
            sync.wait_ge(od1_sem, 32)

        @block.gpsimd
        def _(gpsimd):
            gpsimd.wait_ge(od0_sem, 16)
            gpsimd.wait_ge(od1_sem, 32)

        @block.scalar
        def _(scalar):
            scalar.dma_start(
                out=flat(xall, XP_N, XALL_FLAT), in_=xall_d[:, XP_N:]
            ).then_inc(x1a_sem, 16)
            scalar.dma_start(out=flat(G, GA, G_FLAT), in_=g_d[:, GA:]).then_inc(
                gb_sem, 16
            )
            scalar.dma_start(out=flat(BIAS, 0, O_FLAT), in_=bias_d[:]).then_inc(
                bis_sem, 16
            )
            # dummy op hoists the lazy ACT_TABLE_LOAD off the critical path
            zero_ap = nc.const_aps.aps[(mybir.dt.float32, 0.0)]
            scalar.activation(scratch[:, 0:1], zero_ap, Act.Copy)
            for u in range(6):
                h, c = divmod(u, 3)
                hs = 128 * h
                if u == 0:
                    # split: planes 0..7 as soon as they finish, then 8..14
                    scalar.wait_ge(pe_mid_sem, 1)
                    scalar.activation(
                        Ssb[:, 0:8, c, hs:hs + 128], ps[0][:, 0:8], Act.Copy
                    ).then_inc(act_sem, 1)
                    scalar.wait_ge(pe_sem, 1)
                    scalar.activation(
                        Ssb[:, 8:NV, c, hs:hs + 128], ps[0][:, 8:NV], Act.Copy
                    ).then_inc(act_sem, 1)
                else:
                    scalar.wait_ge(pe_sem, u + 1)
                    scalar.activation(
                        Ssb[:, :, c, hs:hs + 128], ps[u % 2][:], Act.Copy
                    ).then_inc(act_sem, 1)
            scalar.wait_ge(dh1_sem, 1)
            scalar.dma_start(out=out_d[:, 576:768], in_=flat(O, 576, 768)).then_inc(
                od1_sem, 16
            )
            scalar.wait_ge(od1_sem, 32)

        @block.tensor
        def _(tensor):
            pd = list(xall[:].ap[0])
            tensor.wait_ge(bst_sem, 16)
            tensor.wait_ge(xa_sem, 16)
            for u in range(6):
                h, c = divmod(u, 3)
                hs = 128 * h
                tensor.wait_ge(u_sem, c + 1)
                if u >= 2:
                    tensor.wait_ge(act_sem, u)
                for i, (plane, acl, d, st, sp) in enumerate(_EMIT):
                    if acl == 4:
                        rhs = bass.AP(xall, c * 264 + 4 + hs, [pd, [1, 128]])
                    else:
                        rhs = U[:, acl, c, hs:hs + 128]
                    mm = tensor.matmul(
                        ps[u % 2][:, plane],
                        BST[:, d],
                        rhs,
                        start=st,
                        stop=sp,
                        skip_group_check=True,
                    )
                    if u == 0 and i == _MID:
                        mm.then_inc(pe_mid_sem, 1)
                    if i == len(_EMIT) - 1:
                        mm.then_inc(pe_sem, 1)

        @block.vector
        def _(vector):
            pd = list(xall[:].ap[0])
            pdU = list(U[:].ap[0])

            # U column-class sums, per channel; classes {1,3} from xp
            # first (xp rides the sync queue and lands first)
            vector.wait_ge(xa_sem, 16)
            for c in range(C):
                vector.tensor_tensor(
                    bass.AP(U, 768 + c * 256, [pdU, [1536, 2], [1, 256]]),
                    bass.AP(xall, c * 264 + 2, [pd, [-2, 2], [1, 256]]),
                    bass.AP(xall, c * 264 + 6, [pd, [2, 2], [1, 256]]),
                    Alu.add,
                )
            vector.wait_ge(x1a_sem, 16)
            for c in range(C):
                vector.tensor_tensor(
                    bass.AP(U, c * 256, [pdU, [1536, 2], [1, 256]]),
                    bass.AP(xall, XP_N + c * 262 + 2, [pd, [-2, 2], [1, 256]]),
                    bass.AP(xall, XP_N + c * 262 + 4, [pd, [2, 2], [1, 256]]),
                    Alu.add,
                ).then_inc(u_sem, 1)

            def products(lo, hi, c, hs):
                vector.tensor_tensor(
                    P[:, lo:hi, c, hs:hs + 128],
                    Ssb[:, lo:hi, c, hs:hs + 128],
                    G[:, lo:hi, hs:hs + 128],
                    Alu.mult,
                )

            def tree(h):
                hs = 128 * h
                vector.tensor_tensor(
                    P[:, 0:7, :, hs:hs + 128],
                    P[:, 0:7, :, hs:hs + 128],
                    P[:, 7:14, :, hs:hs + 128],
                    Alu.add,
                )
                vector.tensor_tensor(
                    P[:, 0:3, :, hs:hs + 128],
                    P[:, 0:3, :, hs:hs + 128],
                    P[:, 3:6, :, hs:hs + 128],
                    Alu.add,
                )
                p2 = P[:, 2, :, hs:hs + 128]
                vector.tensor_tensor(
                    P[:, 0:2, :, hs:hs + 128],
                    P[:, 0:2, :, hs:hs + 128],
                    bass.AP(
                        P,
                        p2.offset,
                        [list(p2.ap[0]), [4 * C * 256, 2]]
                        + [list(x) for x in p2.ap[1:]],
                    ),
                    Alu.add,
                )
                vector.tensor_tensor(
                    P[:, 0, :, hs:hs + 128],
                    P[:, 0, :, hs:hs + 128],
                    P[:, 1, :, hs:hs + 128],
                    Alu.add,
                )
                vector.tensor_tensor(
                    O[:, h], P[:, 0, :, hs:hs + 128], P[:, 14, :, hs:hs + 128],
                    Alu.add,
                )
                return vector.tensor_tensor(O[:, h], O[:, h], BIAS[:, h], Alu.add)

            vector.wait_ge(ga_sem, 16)
            vector.wait_ge(gb_sem, 16)
            # unit 0 split in two to chase the split evacuation
            vector.wait_ge(act_sem, 1)
            products(0, 8, 0, 0)
            vector.wait_ge(act_sem, 2)
            products(8, NV, 0, 0)
            for u in range(1, 6):
                h, c = divmod(u, 3)
                hs = 128 * h
                vector.wait_ge(act_sem, u + 2)
                products(0, NV, c, hs)
                if u == 2:
                    vector.wait_ge(bis_sem, 16)
                    tree(0).then_inc(dh0_sem, 1)
                if u == 5:
                    tree(1).then_inc(dh1_sem, 1)

    return nc


_PROGRAM_CACHE = {}


def _get_program():
    if "nc" not in _PROGRAM_CACHE:
        import sys

        if "/opt/trn_rl_repo" not in sys.path:
            sys.path.insert(0, "/opt/trn_rl_repo")
        from concourse import bass, mybir

        nc = bass.Bass()
        _PROGRAM_CACHE["nc"] = _build_program(nc, bass, mybir)
    return _PROGRAM_CACHE["nc"]


def _build_bst():
    bst = np.zeros((5, 128, 128), np.float32)  # [d, in row i, out row r]
    for d in range(5):
        for r in range(128):
            for s in ({d, -d} if d else {0}):
                i = r + s
                if i < 0:
                    i = -i  # top reflect
                if i <= 127:
                    bst[d, i, r] += 1.0
    return bst


def _host_prep(x, foa_xy):
    import ml_dtypes

    bf = ml_dtypes.bfloat16
    x = np.asarray(x)
    bst = _build_bst().transpose(1, 0, 2)  # [i, d, r]
    bst_flat = np.ascontiguousarray(bst.reshape(128, BST_FLAT).astype(bf))
    in_maps = []
    for core in range(N_CORES):
        b, half = divmod(core, 2)
        xb = x[b] if half == 0 else x[b][:, ::-1, :]
        xw = xb[:, 0:132, :]
        xpad = np.pad(xw, ((0, 0), (0, 0), (PAD, PAD)), mode="reflect")  # [3,132,264]
        xp = np.ascontiguousarray(xpad[:, 0:128, :].transpose(1, 0, 2)).astype(bf)
        xp1 = np.ascontiguousarray(xpad[:, 0:128, 1:263].transpose(1, 0, 2)).astype(bf)
        xall = np.concatenate(
            [xp.reshape(128, XP_N), xp1.reshape(128, XP1_N)], axis=1
        )

        rp = np.arange(128)
        yy_img = rp if half == 0 else 255 - rp
        yy, xx = np.meshgrid(
            yy_img.astype(np.float64), np.arange(W, dtype=np.float64), indexing="ij"
        )
        fx, fy = float(foa_xy[b, 0]), float(foa_xy[b, 1])
        dist = np.sqrt((xx - fx) ** 2 + (yy - fy) ** 2)
        dn = dist / DIAG
        sigma = (1.0 - dn) * SIGMA_MIN + dn * SIGMA_MAX
        inv2s2 = 1.0 / (2.0 * sigma * sigma)
        base = -dist * np.sqrt(sigma) / (math.pi * sigma ** 4)
        Gf = np.empty((128, NV, 256), np.float32)
        for i, v in enumerate(V_ORD):
            t = v * inv2s2
            Gf[:, i] = base * (1.0 - t) * np.exp(-t)
        Gf[:, 14] = base

        # bias for out rows 124..127: taps at rows 128..131 (outside window)
        rows = xpad[:, 128:132, :].astype(np.float32)  # [3, 4, 264]
        cs = np.zeros((5, 4, C, 256), np.float32)
        for a in range(5):
            for dx in ACL_DX[a]:
                cs[a] += rows[:, :, 4 + dx:4 + dx + 256].transpose(1, 0, 2)
        bias = np.zeros((128, C, 256), np.float32)
        for plane, terms in enumerate(PLANE_TERMS):
            for (d, acl) in terms:
                if d == 0:
                    continue
                for r in range(124, 128):
                    i = r + d
                    if i >= 128:
                        bias[r] += Gf[r, plane][None, :] * cs[acl, i - 128]
        # [128, 2, C, 128] half-major
        bias_t = np.ascontiguousarray(
            bias.reshape(128, C, 2, 128).transpose(0, 2, 1, 3)
        ).astype(bf)

        in_maps.append(
            {
                "xall": np.ascontiguousarray(xall),
                "g": np.ascontiguousarray(Gf.astype(bf).reshape(128, G_FLAT)),
                "bst": bst_flat,
                "bias": bias_t.reshape(128, O_FLAT),
            }
        )
    return in_maps


def _gather(results):
    out = np.empty((B, C, H, W), dtype=np.float32)
    for core in range(N_CORES):
        b, half = divmod(core, 2)
        o = results[core]["out"].astype(np.float32).reshape(128, 2, C, 128)
        o = o.transpose(2, 0, 1, 3).reshape(C, 128, 256)
        if half:
            o = o[:, ::-1, :]
        out[b, :, half * 128:half * 128 + 128, :] = o
    return out


def kernel(x, foa_xy, _trace=False, _tmpdir=None):
    import sys

    if "/opt/trn_rl_repo" not in sys.path:
        sys.path.insert(0, "/opt/trn_rl_repo")
    from concourse.bass_utils import run_bass_kernel_spmd

    nc = _get_program()
    in_maps = _host_prep(np.asarray(x), np.asarray(foa_xy))
    kw = {}
    if _trace:
        kw = dict(trace=True, trace_cores=[], tmpdir=_tmpdir)
    res = run_bass_kernel_spmd(nc, in_maps, list(range(N_CORES)), **kw)
    out = _gather(res.results)
    if _trace:
        return out, res
    return out


# revision 11
# speedup vs baseline: 1.1268x; 1.0051x over previous
"""Adaptive per-pixel LoG 9x9 convolution on 8 TRN2 NeuronCores.

out[b,c,y,x] = sum_{dy,dx in [-4,4]} xpad[b,c,y+dy,x+dx] * K(dx^2+dy^2; p)
K depends on the offset only through r2 = dx^2+dy^2 (15 distinct values)
-> exact rank-15 decomposition  out = sum_v Gp_v * S_v  where S_v are
fixed ring-sum convolutions and Gp_v are host-computed per-pixel weight
planes.

Row-partition layout: 8 cores = 4 batches x 2 row-halves; partition p =
image row p of the half (half1 is vertically flipped by the host so one
SPMD program serves all cores; reflect at the image edge is baked into
the stationary matrices, and the 4 bottom rows' taps that fall outside
the 128-row window arrive as a tiny host-computed bias plane).

Engine split (vs. the all-DVE tile-layout baseline):
- DVE: 6 per-channel column-class sum ops (U), per-unit products vs Gp,
  tree-reduce + bias add. ~14us instead of ~28us.
- PE: all row-band/ring accumulation = 25 banded-stationary matmuls per
  (channel, column-half) unit into PSUM (fp32), FD=128, plane-major
  accumulation groups (interleaved groups mis-accumulate), PSUM
  double-buffered 2x4 banks.
- ACT: evacuates each unit's 15 S-planes PSUM->SBUF bf16 (unit 0 split
  in two so the DVE product stream starts earlier).
- Output DMAed per column-half; the last half rides both queues.
"""

import math

import numpy as np

B, C, H, W = 4, 3, 256, 256
PAD = 4
SIGMA_MIN, SIGMA_MAX = 0.5, 10.0
N_CORES = 8
DIAG = math.sqrt(H * H + W * W)

NV = 15
V_ORD = [1, 4, 9, 16, 5, 10, 17, 13, 20, 25, 2, 8, 18, 32]  # + center v=0 at 14

# plane -> [(d=|dy| class, acl)], acl: 0..3 = |dx| 1..4, 4 = center col
PLANE_TERMS = [
    [(0, 0), (1, 4)],   # v1
    [(0, 1), (2, 4)],   # v4
    [(0, 2), (3, 4)],   # v9
    [(0, 3), (4, 4)],   # v16
    [(1, 1), (2, 0)],   # v5
    [(1, 2), (3, 0)],   # v10
    [(1, 3), (4, 0)],   # v17
    [(2, 2), (3, 1)],   # v13
    [(2, 3), (4, 1)],   # v20
    [(3, 3), (4, 2)],   # v25
    [(1, 0)],           # v2
    [(2, 1)],           # v8
    [(3, 2)],           # v18
    [(4, 3)],           # v32
    [(0, 4)],           # v0 center
]
ACL_DX = [[-1, 1], [-2, 2], [-3, 3], [-4, 4], [0]]

# plane-major matmul emission order: each plane's accumulation group is
# consecutive (interleaved start/stop groups mis-accumulate on HW)
_EMIT = []
for _p, _terms in enumerate(PLANE_TERMS):
    for _i, (_td, _ta) in enumerate(sorted(_terms)):
        _EMIT.append((_p, _ta, _td, _i == 0, _i == len(_terms) - 1))
assert len(_EMIT) == 25
# index of the matmul that completes plane 7 (planes 0..7 evacuate first
# for unit 0)
_MID = sum(len(PLANE_TERMS[p]) for p in range(8)) - 1  # = 15

XP_N = C * 264         # 792
XP1_N = C * 262        # 786
XALL_FLAT = XP_N + XP1_N
G_FLAT = NV * 256      # 3840
BST_FLAT = 5 * 128     # 640
O_FLAT = 2 * C * 128   # 768


def _build_program(nc, bass, mybir):
    bf16 = mybir.dt.bfloat16
    f32 = mybir.dt.float32
    Alu = mybir.AluOpType
    Act = mybir.ActivationFunctionType

    xall_d = nc.declare_dram_parameter("xall", [128, XALL_FLAT], bf16, isOutput=False)
    g_d = nc.declare_dram_parameter("g", [128, G_FLAT], bf16, isOutput=False)
    bst_d = nc.declare_dram_parameter("bst", [128, BST_FLAT], bf16, isOutput=False)
    bias_d = nc.declare_dram_parameter("bias", [128, O_FLAT], bf16, isOutput=False)
    out_d = nc.declare_dram_parameter("out", [128, O_FLAT], bf16, isOutput=True)

    xa_sem = nc.alloc_semaphore("xa_sem")      # xp half (sync queue)
    x1a_sem = nc.alloc_semaphore("x1a_sem")    # xp1 half (scalar queue)
    bst_sem = nc.alloc_semaphore("bst_sem")
    ga_sem = nc.alloc_semaphore("ga_sem")
    gb_sem = nc.alloc_semaphore("gb_sem")
    bis_sem = nc.alloc_semaphore("bis_sem")
    u_sem = nc.alloc_semaphore("u_sem")
    pe_mid_sem = nc.alloc_semaphore("pe_mid_sem")
    pe_sem = nc.alloc_semaphore("pe_sem")
    act_sem = nc.alloc_semaphore("act_sem")
    dh0_sem = nc.alloc_semaphore("dh0_sem")
    dh1_sem = nc.alloc_semaphore("dh1_sem")
    od0_sem = nc.alloc_semaphore("od0_sem")
    od1_sem = nc.alloc_semaphore("od1_sem")

    xall = nc.alloc_sbuf_tensor("s_xall", [128, XALL_FLAT], bf16)
    U = nc.alloc_sbuf_tensor("U", [128, 4, C, 256], bf16)
    Ssb = nc.alloc_sbuf_tensor("Ssb", [128, NV, C, 256], bf16)
    P = nc.alloc_sbuf_tensor("P", [128, NV, C, 256], bf16)
    G = nc.alloc_sbuf_tensor("G", [128, NV, 256], bf16)
    BST = nc.alloc_sbuf_tensor("BST", [128, 5, 128], bf16)
    BIAS = nc.alloc_sbuf_tensor("BIAS", [128, 2, C, 128], bf16)
    O = nc.alloc_sbuf_tensor("O", [128, 2, C, 128], bf16)
    scratch = nc.alloc_sbuf_tensor("scratch", [128, 2], bf16)

    ps = [
        nc.alloc_psum_tensor("ps0", [128, NV, 128], f32),
        nc.alloc_psum_tensor("ps1", [128, NV, 128], f32),
    ]

    GA = 1920

    with nc.Block(no_gpsimd_drain=True) as block:
        def flat(t, lo, hi):
            a = t[:]
            return bass.AP(t, lo, [list(a.ap[0]), [1, hi - lo]])

        pd_xall = None  # filled below

        @block.sync
        def _(sync):
            sync.dma_start(out=flat(xall, 0, XP_N), in_=xall_d[:, 0:XP_N]).then_inc(
                xa_sem, 16
            )
            sync.dma_start(out=flat(G, 0, GA), in_=g_d[:, 0:GA]).then_inc(ga_sem, 16)
            sync.wait_ge(dh0_sem, 1)
            sync.dma_start(out=out_d[:, 0:384], in_=flat(O, 0, 384)).then_inc(
                od0_sem, 16
            )
            sync.wait_ge(dh1_sem, 1)
            sync.dma_start(out=out_d[:, 384:576], in_=flat(O, 384, 576)).then_inc(
                od1_sem, 16
            )
            sync.wait_ge(od0_sem, 16)
            sync.wait_ge(od1_sem, 32)

        @block.gpsimd
        def _(gpsimd):
            gpsimd.wait_ge(od0_sem, 16)
            gpsimd.wait_ge(od1_sem, 32)

        @block.scalar
        def _(scalar):
            # bst first: it gates the PE start and is small; slot-2 spots
            # showed multi-microsecond arrival variance across cores
            scalar.dma_start(out=flat(BST, 0, BST_FLAT), in_=bst_d[:]).then_inc(
                bst_sem, 16
            )
            scalar.dma_start(
                out=flat(xall, XP_N, XALL_FLAT), in_=xall_d[:, XP_N:]
            ).then_inc(x1a_sem, 16)
            scalar.dma_start(out=flat(G, GA, G_FLAT), in_=g_d[:, GA:]).then_inc(
                gb_sem, 16
            )
            scalar.dma_start(out=flat(BIAS, 0, O_FLAT), in_=bias_d[:]).then_inc(
                bis_sem, 16
            )
            # dummy op hoists the lazy ACT_TABLE_LOAD off the critical path
            zero_ap = nc.const_aps.aps[(mybir.dt.float32, 0.0)]
            scalar.activation(scratch[:, 0:1], zero_ap, Act.Copy)
            for u in range(6):
                h, c = divmod(u, 3)
                hs = 128 * h
                if u == 0:
                    # split: planes 0..7 as soon as they finish, then 8..14
                    scalar.wait_ge(pe_mid_sem, 1)
                    scalar.activation(
                        Ssb[:, 0:8, c, hs:hs + 128], ps[0][:, 0:8], Act.Copy
                    ).then_inc(act_sem, 1)
                    scalar.wait_ge(pe_sem, 1)
                    scalar.activation(
                        Ssb[:, 8:NV, c, hs:hs + 128], ps[0][:, 8:NV], Act.Copy
                    ).then_inc(act_sem, 1)
                else:
                    scalar.wait_ge(pe_sem, u + 1)
                    scalar.activation(
                        Ssb[:, :, c, hs:hs + 128], ps[u % 2][:], Act.Copy
                    ).then_inc(act_sem, 1)
            scalar.wait_ge(dh1_sem, 1)
            scalar.dma_start(out=out_d[:, 576:768], in_=flat(O, 576, 768)).then_inc(
                od1_sem, 16
            )
            scalar.wait_ge(od1_sem, 32)

        @block.tensor
        def _(tensor):
            pd = list(xall[:].ap[0])
            tensor.wait_ge(bst_sem, 16)
            tensor.wait_ge(xa_sem, 16)
            for u in range(6):
                h, c = divmod(u, 3)
                hs = 128 * h
                tensor.wait_ge(u_sem, c + 1)
                if u >= 2:
                    tensor.wait_ge(act_sem, u)
                for i, (plane, acl, d, st, sp) in enumerate(_EMIT):
                    if acl == 4:
                        rhs = bass.AP(xall, c * 264 + 4 + hs, [pd, [1, 128]])
                    else:
                        rhs = U[:, acl, c, hs:hs + 128]
                    mm = tensor.matmul(
                        ps[u % 2][:, plane],
                        BST[:, d],
                        rhs,
                        start=st,
                        stop=sp,
                        skip_group_check=True,
                    )
                    if u == 0 and i == _MID:
                        mm.then_inc(pe_mid_sem, 1)
                    if i == len(_EMIT) - 1:
                        mm.then_inc(pe_sem, 1)

        @block.vector
        def _(vector):
            pd = list(xall[:].ap[0])
            pdU = list(U[:].ap[0])

            # U column-class sums, per channel; classes {1,3} from xp
            # first (xp rides the sync queue and lands first)
            vector.wait_ge(xa_sem, 16)
            for c in range(C):
                vector.tensor_tensor(
                    bass.AP(U, 768 + c * 256, [pdU, [1536, 2], [1, 256]]),
                    bass.AP(xall, c * 264 + 2, [pd, [-2, 2], [1, 256]]),
                    bass.AP(xall, c * 264 + 6, [pd, [2, 2], [1, 256]]),
                    Alu.add,
                )
            vector.wait_ge(x1a_sem, 16)
            for c in range(C):
                vector.tensor_tensor(
                    bass.AP(U, c * 256, [pdU, [1536, 2], [1, 256]]),
                    bass.AP(xall, XP_N + c * 262 + 2, [pd, [-2, 2], [1, 256]]),
                    bass.AP(xall, XP_N + c * 262 + 4, [pd, [2, 2], [1, 256]]),
                    Alu.add,
                ).then_inc(u_sem, 1)

            def products(lo, hi, c, hs):
                vector.tensor_tensor(
                    P[:, lo:hi, c, hs:hs + 128],
                    Ssb[:, lo:hi, c, hs:hs + 128],
                    G[:, lo:hi, hs:hs + 128],
                    Alu.mult,
                )

            def tree_t1(h, cs):
                hs = 128 * h
                vector.tensor_tensor(
                    P[:, 0:7, cs, hs:hs + 128],
                    P[:, 0:7, cs, hs:hs + 128],
                    P[:, 7:14, cs, hs:hs + 128],
                    Alu.add,
                )

            def tree_rest(h):
                hs = 128 * h
                vector.tensor_tensor(
                    P[:, 0:3, :, hs:hs + 128],
                    P[:, 0:3, :, hs:hs + 128],
                    P[:, 3:6, :, hs:hs + 128],
                    Alu.add,
                )
                p2 = P[:, 2, :, hs:hs + 128]
                vector.tensor_tensor(
                    P[:, 0:2, :, hs:hs + 128],
                    P[:, 0:2, :, hs:hs + 128],
                    bass.AP(
                        P,
                        p2.offset,
                        [list(p2.ap[0]), [4 * C * 256, 2]]
                        + [list(x) for x in p2.ap[1:]],
                    ),
                    Alu.add,
                )
                vector.tensor_tensor(
                    P[:, 0, :, hs:hs + 128],
                    P[:, 0, :, hs:hs + 128],
                    P[:, 1, :, hs:hs + 128],
                    Alu.add,
                )
                vector.tensor_tensor(
                    O[:, h], P[:, 0, :, hs:hs + 128], P[:, 14, :, hs:hs + 128],
                    Alu.add,
                )
                return vector.tensor_tensor(O[:, h], O[:, h], BIAS[:, h], Alu.add)

            vector.wait_ge(ga_sem, 16)
            vector.wait_ge(gb_sem, 16)
            # unit 0 split in two to chase the split evacuation
            vector.wait_ge(act_sem, 1)
            products(0, 8, 0, 0)
            vector.wait_ge(act_sem, 2)
            products(8, NV, 0, 0)
            for u in range(1, 6):
                h, c = divmod(u, 3)
                hs = 128 * h
                vector.wait_ge(act_sem, u + 2)
                products(0, NV, c, hs)
                if u == 2:
                    vector.wait_ge(bis_sem, 16)
                    tree_t1(0, slice(0, C))
                    tree_rest(0).then_inc(dh0_sem, 1)
                elif u in (3, 4):
                    # hoist half1's first tree level off the tail, one
                    # channel at a time as its products land
                    tree_t1(1, slice(u - 3, u - 2))
                elif u == 5:
                    tree_t1(1, slice(2, 3))
                    tree_rest(1).then_inc(dh1_sem, 1)

    return nc


_PROGRAM_CACHE = {}


def _get_program():
    if "nc" not in _PROGRAM_CACHE:
        import sys

        if "/opt/trn_rl_repo" not in sys.path:
            sys.path.insert(0, "/opt/trn_rl_repo")
        from concourse import bass, mybir

        nc = bass.Bass()
        _PROGRAM_CACHE["nc"] = _build_program(nc, bass, mybir)
    return _PROGRAM_CACHE["nc"]


def _build_bst():
    bst = np.zeros((5, 128, 128), np.float32)  # [d, in row i, out row r]
    for d in range(5):
        for r in range(128):
            for s in ({d, -d} if d else {0}):
                i = r + s
                if i < 0:
                    i = -i  # top reflect
                if i <= 127:
                    bst[d, i, r] += 1.0
    return bst


def _host_prep(x, foa_xy):
    import ml_dtypes

    bf = ml_dtypes.bfloat16
    x = np.asarray(x)
    bst = _build_bst().transpose(1, 0, 2)  # [i, d, r]
    bst_flat = np.ascontiguousarray(bst.reshape(128, BST_FLAT).astype(bf))
    in_maps = []
    for core in range(N_CORES):
        b, half = divmod(core, 2)
        xb = x[b] if half == 0 else x[b][:, ::-1, :]
        xw = xb[:, 0:132, :]
        xpad = np.pad(xw, ((0, 0), (0, 0), (PAD, PAD)), mode="reflect")  # [3,132,264]
        xp = np.ascontiguousarray(xpad[:, 0:128, :].transpose(1, 0, 2)).astype(bf)
        xp1 = np.ascontiguousarray(xpad[:, 0:128, 1:263].transpose(1, 0, 2)).astype(bf)
        xall = np.concatenate(
            [xp.reshape(128, XP_N), xp1.reshape(128, XP1_N)], axis=1
        )

        rp = np.arange(128)
        yy_img = rp if half == 0 else 255 - rp
        yy, xx = np.meshgrid(
            yy_img.astype(np.float64), np.arange(W, dtype=np.float64), indexing="ij"
        )
        fx, fy = float(foa_xy[b, 0]), float(foa_xy[b, 1])
        dist = np.sqrt((xx - fx) ** 2 + (yy - fy) ** 2)
        dn = dist / DIAG
        sigma = (1.0 - dn) * SIGMA_MIN + dn * SIGMA_MAX
        inv2s2 = 1.0 / (2.0 * sigma * sigma)
        base = -dist * np.sqrt(sigma) / (math.pi * sigma ** 4)
        Gf = np.empty((128, NV, 256), np.float32)
        for i, v in enumerate(V_ORD):
            t = v * inv2s2
            Gf[:, i] = base * (1.0 - t) * np.exp(-t)
        Gf[:, 14] = base

        # bias for out rows 124..127: taps at rows 128..131 (outside window)
        rows = xpad[:, 128:132, :].astype(np.float32)  # [3, 4, 264]
        cs = np.zeros((5, 4, C, 256), np.float32)
        for a in range(5):
            for dx in ACL_DX[a]:
                cs[a] += rows[:, :, 4 + dx:4 + dx + 256].transpose(1, 0, 2)
        bias = np.zeros((128, C, 256), np.float32)
        for plane, terms in enumerate(PLANE_TERMS):
            for (d, acl) in terms:
                if d == 0:
                    continue
                for r in range(124, 128):
                    i = r + d
                    if i >= 128:
                        bias[r] += Gf[r, plane][None, :] * cs[acl, i - 128]
        # [128, 2, C, 128] half-major
        bias_t = np.ascontiguousarray(
            bias.reshape(128, C, 2, 128).transpose(0, 2, 1, 3)
        ).astype(bf)

        in_maps.append(
            {
                "xall": np.ascontiguousarray(xall),
                "g": np.ascontiguousarray(Gf.astype(bf).reshape(128, G_FLAT)),
                "bst": bst_flat,
                "bias": bias_t.reshape(128, O_FLAT),
            }
        )
    return in_maps


def _gather(results):
    out = np.empty((B, C, H, W), dtype=np.float32)
    for core in range(N_CORES):
        b, half = divmod(core, 2)
        o = results[core]["out"].astype(np.float32).reshape(128, 2, C, 128)
        o = o.transpose(2, 0, 1, 3).reshape(C, 128, 256)
        if half:
            o = o[:, ::-1, :]
        out[b, :, half * 128:half * 128 + 128, :] = o
    return out


def kernel(x, foa_xy, _trace=False, _tmpdir=None):
    import sys

    if "/opt/trn_rl_repo" not in sys.path:
        sys.path.insert(0, "/opt/trn_rl_repo")
    from concourse.bass_utils import run_bass_kernel_spmd

    nc = _get_program()
    in_maps = _host_prep(np.asarray(x), np.asarray(foa_xy))
    kw = {}
    if _trace:
        kw = dict(trace=True, trace_cores=[], tmpdir=_tmpdir)
    res = run_bass_kernel_spmd(nc, in_maps, list(range(N_CORES)), **kw)
    out = _gather(res.results)
    if _trace:
        return out, res
    return out


# revision 19
# speedup vs baseline: 1.2138x; 1.0772x over previous
"""Adaptive per-pixel LoG 9x9 convolution on 8 TRN2 NeuronCores.

out[b,c,y,x] = sum_{dy,dx in [-4,4]} xpad[b,c,y+dy,x+dx] * K(dx^2+dy^2; p)
K depends on the offset only through r2 = dx^2+dy^2 (15 distinct values)
-> exact rank-15 decomposition  out = sum_v Gp_v * S_v  where S_v are
fixed ring-sum convolutions and Gp_v are host-computed per-pixel weight
planes.

Row-partition layout: 8 cores = 4 batches x 2 row-halves; partition p =
image row p of the half (half1 is vertically flipped by the host so one
SPMD program serves all cores; reflect at the image edge is baked into
the stationary matrices, and the 4 bottom rows' taps that fall outside
the 128-row window arrive as a tiny host-computed bias plane).

Engine split (vs. the all-DVE tile-layout baseline):
- DVE: 6 per-channel column-class sum ops (U), per-unit products vs Gp,
  tree-reduce + bias add. ~14us instead of ~28us.
- PE: all row-band/ring accumulation = 25 banded-stationary matmuls per
  (channel, column-half) unit into PSUM (fp32), FD=128, plane-major
  accumulation groups (interleaved groups mis-accumulate), PSUM
  double-buffered 2x4 banks.
- ACT: evacuates each unit's 15 S-planes PSUM->SBUF bf16 (unit 0 split
  in two so the DVE product stream starts earlier).
- Output DMAed per column-half; the last half rides both queues.
"""

import math

import numpy as np

B, C, H, W = 4, 3, 256, 256
PAD = 4
SIGMA_MIN, SIGMA_MAX = 0.5, 10.0
N_CORES = 8
DIAG = math.sqrt(H * H + W * W)

NV = 15
V_ORD = [1, 4, 9, 16, 5, 10, 17, 13, 20, 25, 2, 8, 18, 32]  # + center v=0 at 14

# plane -> [(d=|dy| class, acl)], acl: 0..3 = |dx| 1..4, 4 = center col
PLANE_TERMS = [
    [(0, 0), (1, 4)],   # v1
    [(0, 1), (2, 4)],   # v4
    [(0, 2), (3, 4)],   # v9
    [(0, 3), (4, 4)],   # v16
    [(1, 1), (2, 0)],   # v5
    [(1, 2), (3, 0)],   # v10
    [(1, 3), (4, 0)],   # v17
    [(2, 2), (3, 1)],   # v13
    [(2, 3), (4, 1)],   # v20
    [(3, 3), (4, 2)],   # v25
    [(1, 0)],           # v2
    [(2, 1)],           # v8
    [(3, 2)],           # v18
    [(4, 3)],           # v32
    [(0, 4)],           # v0 center
]
ACL_DX = [[-1, 1], [-2, 2], [-3, 3], [-4, 4], [0]]

# plane-major matmul emission order: each plane's accumulation group is
# consecutive (interleaved start/stop groups mis-accumulate on HW)
_EMIT = []
for _p, _terms in enumerate(PLANE_TERMS):
    for _i, (_td, _ta) in enumerate(sorted(_terms)):
        _EMIT.append((_p, _ta, _td, _i == 0, _i == len(_terms) - 1))
assert len(_EMIT) == 25
# index of the matmul that completes plane 7 (planes 0..7 evacuate first
# for unit 0)
_MID = sum(len(PLANE_TERMS[p]) for p in range(8)) - 1  # = 15

XP_N = C * 264         # 792
G_FLAT = NV * 256      # 3840
BST_FLAT = 5 * 128     # 640
O_FLAT = 2 * C * 128   # 768


def _build_program(nc, bass, mybir):
    bf16 = mybir.dt.bfloat16
    f32 = mybir.dt.float32
    Alu = mybir.AluOpType
    Act = mybir.ActivationFunctionType

    xall_d = nc.declare_dram_parameter("xall", [128, XP_N], bf16, isOutput=False)
    g_d = nc.declare_dram_parameter("g", [128, G_FLAT], bf16, isOutput=False)
    bst_d = nc.declare_dram_parameter("bst", [128, BST_FLAT], bf16, isOutput=False)
    bias_d = nc.declare_dram_parameter("bias", [128, O_FLAT], bf16, isOutput=False)
    out_d = nc.declare_dram_parameter("out", [128, O_FLAT], bf16, isOutput=True)

    xa_sem = nc.alloc_semaphore("xa_sem")      # xp channel 0 (sync queue)
    x1a_sem = nc.alloc_semaphore("x1a_sem")    # xp channels 1,2 (sync queue)
    bst_sem = nc.alloc_semaphore("bst_sem")
    ga_sem = nc.alloc_semaphore("ga_sem")
    gb_sem = nc.alloc_semaphore("gb_sem")
    bis_sem = nc.alloc_semaphore("bis_sem")
    u_sem = nc.alloc_semaphore("u_sem")
    pe_mid_sem = nc.alloc_semaphore("pe_mid_sem")
    pe_sem = nc.alloc_semaphore("pe_sem")
    act_sem = nc.alloc_semaphore("act_sem")
    dh0_sem = nc.alloc_semaphore("dh0_sem")
    dh1_sem = nc.alloc_semaphore("dh1_sem")
    od0_sem = nc.alloc_semaphore("od0_sem")
    od1_sem = nc.alloc_semaphore("od1_sem")

    xall = nc.alloc_sbuf_tensor("s_xall", [128, XP_N], bf16)
    U = nc.alloc_sbuf_tensor("U", [128, 4, C, 256], bf16)
    Ssb = nc.alloc_sbuf_tensor("Ssb", [128, NV, C, 256], bf16)
    P = nc.alloc_sbuf_tensor("P", [128, NV, C, 256], bf16)
    G = nc.alloc_sbuf_tensor("G", [128, NV, 256], bf16)
    BST = nc.alloc_sbuf_tensor("BST", [128, 5, 128], bf16)
    BIAS = nc.alloc_sbuf_tensor("BIAS", [128, 2, C, 128], bf16)
    O = nc.alloc_sbuf_tensor("O", [128, 2, C, 128], bf16)
    scratch = nc.alloc_sbuf_tensor("scratch", [128, 2], bf16)

    ps = [
        nc.alloc_psum_tensor("ps0", [128, NV, 128], f32),
        nc.alloc_psum_tensor("ps1", [128, NV, 128], f32),
    ]

    GA = 1920

    with nc.Block(no_gpsimd_drain=True) as block:
        def flat(t, lo, hi):
            a = t[:]
            return bass.AP(t, lo, [list(a.ap[0]), [1, hi - lo]])

        pd_xall = None  # filled below

        @block.sync
        def _(sync):
            # channel 0 first so the U chain (and PE unit 0) starts early
            sync.dma_start(out=flat(xall, 0, 264), in_=xall_d[:, 0:264]).then_inc(
                xa_sem, 16
            )
            sync.dma_start(out=flat(xall, 264, XP_N), in_=xall_d[:, 264:]).then_inc(
                x1a_sem, 16
            )
            sync.dma_start(out=flat(G, 0, GA), in_=g_d[:, 0:GA]).then_inc(ga_sem, 16)
            sync.wait_ge(dh0_sem, 1)
            sync.dma_start(out=out_d[:, 0:384], in_=flat(O, 0, 384)).then_inc(
                od0_sem, 16
            )
            sync.wait_ge(dh1_sem, 1)
            sync.dma_start(out=out_d[:, 384:576], in_=flat(O, 384, 576)).then_inc(
                od1_sem, 16
            )
            sync.wait_ge(od0_sem, 16)
            sync.wait_ge(od1_sem, 32)

        @block.gpsimd
        def _(gpsimd):
            gpsimd.wait_ge(od0_sem, 16)
            gpsimd.wait_ge(od1_sem, 32)

        @block.scalar
        def _(scalar):
            # bst first: it gates the PE start and is small; slot-2 spots
            # showed multi-microsecond arrival variance across cores
            scalar.dma_start(out=flat(BST, 0, BST_FLAT), in_=bst_d[:]).then_inc(
                bst_sem, 16
            )
            scalar.dma_start(out=flat(G, GA, G_FLAT), in_=g_d[:, GA:]).then_inc(
                gb_sem, 16
            )
            scalar.dma_start(out=flat(BIAS, 0, O_FLAT), in_=bias_d[:]).then_inc(
                bis_sem, 16
            )
            # dummy op hoists the lazy ACT_TABLE_LOAD off the critical path
            zero_ap = nc.const_aps.aps[(mybir.dt.float32, 0.0)]
            scalar.activation(scratch[:, 0:1], zero_ap, Act.Copy)
            for u in range(6):
                h, c = divmod(u, 3)
                hs = 128 * h
                if u == 0:
                    # split: planes 0..7 as soon as they finish, then 8..14
                    scalar.wait_ge(pe_mid_sem, 1)
                    scalar.activation(
                        Ssb[:, 0:8, c, hs:hs + 128], ps[0][:, 0:8], Act.Copy
                    ).then_inc(act_sem, 1)
                    scalar.wait_ge(pe_sem, 1)
                    scalar.activation(
                        Ssb[:, 8:NV, c, hs:hs + 128], ps[0][:, 8:NV], Act.Copy
                    ).then_inc(act_sem, 1)
                else:
                    scalar.wait_ge(pe_sem, u + 1)
                    scalar.activation(
                        Ssb[:, :, c, hs:hs + 128], ps[u % 2][:], Act.Copy
                    ).then_inc(act_sem, 1)
            scalar.wait_ge(dh1_sem, 1)
            scalar.dma_start(out=out_d[:, 576:768], in_=flat(O, 576, 768)).then_inc(
                od1_sem, 16
            )
            scalar.wait_ge(od1_sem, 32)

        @block.tensor
        def _(tensor):
            pd = list(xall[:].ap[0])
            tensor.wait_ge(bst_sem, 16)
            tensor.wait_ge(xa_sem, 16)
            for u in range(6):
                h, c = divmod(u, 3)
                hs = 128 * h
                tensor.wait_ge(u_sem, c + 1)
                if u >= 2:
                    tensor.wait_ge(act_sem, u)
                for i, (plane, acl, d, st, sp) in enumerate(_EMIT):
                    if acl == 4:
                        rhs = bass.AP(xall, c * 264 + 4 + hs, [pd, [1, 128]])
                    else:
                        rhs = U[:, acl, c, hs:hs + 128]
                    mm = tensor.matmul(
                        ps[u % 2][:, plane],
                        BST[:, d],
                        rhs,
                        start=st,
                        stop=sp,
                        skip_group_check=True,
                    )
                    if u == 0 and i == _MID:
                        mm.then_inc(pe_mid_sem, 1)
                    if i == len(_EMIT) - 1:
                        mm.then_inc(pe_sem, 1)

        @block.vector
        def _(vector):
            pd = list(xall[:].ap[0])
            pdU = list(U[:].ap[0])

            # U column-class sums, per channel: even classes {1,3} hit
            # DVE 2x mode; odd classes {0,2} run at 1x (odd element
            # offsets) — still cheaper than shipping an aligned copy
            vector.wait_ge(xa_sem, 16)
            for c in range(C):
                if c == 1:
                    vector.wait_ge(x1a_sem, 16)
                vector.tensor_tensor(
                    bass.AP(U, 768 + c * 256, [pdU, [1536, 2], [1, 256]]),
                    bass.AP(xall, c * 264 + 2, [pd, [-2, 2], [1, 256]]),
                    bass.AP(xall, c * 264 + 6, [pd, [2, 2], [1, 256]]),
                    Alu.add,
                )
                vector.tensor_tensor(
                    bass.AP(U, c * 256, [pdU, [1536, 2], [1, 256]]),
                    bass.AP(xall, c * 264 + 3, [pd, [-2, 2], [1, 256]]),
                    bass.AP(xall, c * 264 + 5, [pd, [2, 2], [1, 256]]),
                    Alu.add,
                ).then_inc(u_sem, 1)

            def products(lo, hi, c, hs):
                vector.tensor_tensor(
                    P[:, lo:hi, c, hs:hs + 128],
                    Ssb[:, lo:hi, c, hs:hs + 128],
                    G[:, lo:hi, hs:hs + 128],
                    Alu.mult,
                )

            def tree_t1(h, cs):
                hs = 128 * h
                vector.tensor_tensor(
                    P[:, 0:7, cs, hs:hs + 128],
                    P[:, 0:7, cs, hs:hs + 128],
                    P[:, 7:14, cs, hs:hs + 128],
                    Alu.add,
                )

            def tree_rest(h):
                hs = 128 * h
                vector.tensor_tensor(
                    P[:, 0:3, :, hs:hs + 128],
                    P[:, 0:3, :, hs:hs + 128],
                    P[:, 3:6, :, hs:hs + 128],
                    Alu.add,
                )
                p2 = P[:, 2, :, hs:hs + 128]
                vector.tensor_tensor(
                    P[:, 0:2, :, hs:hs + 128],
                    P[:, 0:2, :, hs:hs + 128],
                    bass.AP(
                        P,
                        p2.offset,
                        [list(p2.ap[0]), [4 * C * 256, 2]]
                        + [list(x) for x in p2.ap[1:]],
                    ),
                    Alu.add,
                )
                vector.tensor_tensor(
                    P[:, 0, :, hs:hs + 128],
                    P[:, 0, :, hs:hs + 128],
                    P[:, 1, :, hs:hs + 128],
                    Alu.add,
                )
                vector.tensor_tensor(
                    O[:, h], P[:, 0, :, hs:hs + 128], P[:, 14, :, hs:hs + 128],
                    Alu.add,
                )
                return vector.tensor_tensor(O[:, h], O[:, h], BIAS[:, h], Alu.add)

            vector.wait_ge(ga_sem, 16)
            vector.wait_ge(gb_sem, 16)
            # unit 0 split in two to chase the split evacuation
            vector.wait_ge(act_sem, 1)
            products(0, 8, 0, 0)
            vector.wait_ge(act_sem, 2)
            products(8, NV, 0, 0)
            for u in range(1, 6):
                h, c = divmod(u, 3)
                hs = 128 * h
                vector.wait_ge(act_sem, u + 2)
                products(0, NV, c, hs)
                if u == 2:
                    vector.wait_ge(bis_sem, 16)
                    tree_t1(0, slice(0, C))
                    tree_rest(0).then_inc(dh0_sem, 1)
                elif u in (3, 4):
                    # hoist half1's first tree level off the tail, one
                    # channel at a time as its products land
                    tree_t1(1, slice(u - 3, u - 2))
                elif u == 5:
                    tree_t1(1, slice(2, 3))
                    tree_rest(1).then_inc(dh1_sem, 1)

    return nc


_PROGRAM_CACHE = {}


def _get_program():
    if "nc" not in _PROGRAM_CACHE:
        import sys

        if "/opt/trn_rl_repo" not in sys.path:
            sys.path.insert(0, "/opt/trn_rl_repo")
        from concourse import bass, mybir

        nc = bass.Bass()
        _PROGRAM_CACHE["nc"] = _build_program(nc, bass, mybir)
    return _PROGRAM_CACHE["nc"]


def _build_bst():
    bst = np.zeros((5, 128, 128), np.float32)  # [d, in row i, out row r]
    for d in range(5):
        for r in range(128):
            for s in ({d, -d} if d else {0}):
                i = r + s
                if i < 0:
                    i = -i  # top reflect
                if i <= 127:
                    bst[d, i, r] += 1.0
    return bst


def _host_prep(x, foa_xy):
    import ml_dtypes

    bf = ml_dtypes.bfloat16
    x = np.asarray(x)
    bst = _build_bst().transpose(1, 0, 2)  # [i, d, r]
    bst_flat = np.ascontiguousarray(bst.reshape(128, BST_FLAT).astype(bf))
    in_maps = []
    for core in range(N_CORES):
        b, half = divmod(core, 2)
        xb = x[b] if half == 0 else x[b][:, ::-1, :]
        xw = xb[:, 0:132, :]
        xpad = np.pad(xw, ((0, 0), (0, 0), (PAD, PAD)), mode="reflect")  # [3,132,264]
        xp = np.ascontiguousarray(xpad[:, 0:128, :].transpose(1, 0, 2)).astype(bf)
        xall = xp.reshape(128, XP_N)

        rp = np.arange(128)
        yy_img = rp if half == 0 else 255 - rp
        yy, xx = np.meshgrid(
            yy_img.astype(np.float64), np.arange(W, dtype=np.float64), indexing="ij"
        )
        fx, fy = float(foa_xy[b, 0]), float(foa_xy[b, 1])
        dist = np.sqrt((xx - fx) ** 2 + (yy - fy) ** 2)
        dn = dist / DIAG
        sigma = (1.0 - dn) * SIGMA_MIN + dn * SIGMA_MAX
        inv2s2 = 1.0 / (2.0 * sigma * sigma)
        base = -dist * np.sqrt(sigma) / (math.pi * sigma ** 4)
        Gf = np.empty((128, NV, 256), np.float32)
        for i, v in enumerate(V_ORD):
            t = v * inv2s2
            Gf[:, i] = base * (1.0 - t) * np.exp(-t)
        Gf[:, 14] = base

        # bias for out rows 124..127: taps at rows 128..131 (outside window)
        rows = xpad[:, 128:132, :].astype(np.float32)  # [3, 4, 264]
        cs = np.zeros((5, 4, C, 256), np.float32)
        for a in range(5):
            for dx in ACL_DX[a]:
                cs[a] += rows[:, :, 4 + dx:4 + dx + 256].transpose(1, 0, 2)
        bias = np.zeros((128, C, 256), np.float32)
        for plane, terms in enumerate(PLANE_TERMS):
            for (d, acl) in terms:
                if d == 0:
                    continue
                for r in range(124, 128):
                    i = r + d
                    if i >= 128:
                        bias[r] += Gf[r, plane][None, :] * cs[acl, i - 128]
        # [128, 2, C, 128] half-major
        bias_t = np.ascontiguousarray(
            bias.reshape(128, C, 2, 128).transpose(0, 2, 1, 3)
        ).astype(bf)

        in_maps.append(
            {
                "xall": np.ascontiguousarray(xall),
                "g": np.ascontiguousarray(Gf.astype(bf).reshape(128, G_FLAT)),
                "bst": bst_flat,
                "bias": bias_t.reshape(128, O_FLAT),
            }
        )
    return in_maps


def _gather(results):
    out = np.empty((B, C, H, W), dtype=np.float32)
    for core in range(N_CORES):
        b, half = divmod(core, 2)
        o = results[core]["out"].astype(np.float32).reshape(128, 2, C, 128)
        o = o.transpose(2, 0, 1, 3).reshape(C, 128, 256)
        if half:
            o = o[:, ::-1, :]
        out[b, :, half * 128:half * 128 + 128, :] = o
    return out


def kernel(x, foa_xy, _trace=False, _tmpdir=None):
    import sys

    if "/opt/trn_rl_repo" not in sys.path:
        sys.path.insert(0, "/opt/trn_rl_repo")
    from concourse.bass_utils import run_bass_kernel_spmd

    nc = _get_program()
    in_maps = _host_prep(np.asarray(x), np.asarray(foa_xy))
    kw = {}
    if _trace:
        kw = dict(trace=True, trace_cores=[], tmpdir=_tmpdir)
    res = run_bass_kernel_spmd(nc, in_maps, list(range(N_CORES)), **kw)
    out = _gather(res.results)
    if _trace:
        return out, res
    return out


# revision 25
# speedup vs baseline: 1.2258x; 1.0098x over previous
"""Adaptive per-pixel LoG 9x9 convolution on 8 TRN2 NeuronCores.

out[b,c,y,x] = sum_{dy,dx in [-4,4]} xpad[b,c,y+dy,x+dx] * K(dx^2+dy^2; p)
K depends on the offset only through r2 = dx^2+dy^2 (15 distinct values)
-> exact rank-15 decomposition  out = sum_v Gp_v * S_v  where S_v are
fixed ring-sum convolutions and Gp_v are host-computed per-pixel weight
planes.

Row-partition layout: 8 cores = 4 batches x 2 row-halves; partition p =
image row p of the half (half1 is vertically flipped by the host so one
SPMD program serves all cores; reflect at the image edge is baked into
the stationary matrices, and the 4 bottom rows' taps that fall outside
the 128-row window arrive as a tiny host-computed bias plane).

Engine split (vs. the all-DVE tile-layout baseline):
- DVE: 6 per-channel column-class sum ops (U), per-unit products vs Gp,
  tree-reduce + bias add. ~14us instead of ~28us.
- PE: all row-band/ring accumulation = 25 banded-stationary matmuls per
  (channel, column-half) unit into PSUM (fp32), FD=128, plane-major
  accumulation groups (interleaved groups mis-accumulate), PSUM
  double-buffered 2x4 banks.
- ACT: evacuates each unit's 15 S-planes PSUM->SBUF bf16 (unit 0 split
  in two so the DVE product stream starts earlier).
- Output DMAed per column-half; the last half rides both queues.
"""

import math

import numpy as np

B, C, H, W = 4, 3, 256, 256
PAD = 4
SIGMA_MIN, SIGMA_MAX = 0.5, 10.0
N_CORES = 8
DIAG = math.sqrt(H * H + W * W)

NV = 15
V_ORD = [1, 4, 9, 16, 5, 10, 17, 13, 20, 25, 2, 8, 18, 32]  # + center v=0 at 14

# plane -> [(d=|dy| class, acl)], acl: 0..3 = |dx| 1..4, 4 = center col
PLANE_TERMS = [
    [(0, 0), (1, 4)],   # v1
    [(0, 1), (2, 4)],   # v4
    [(0, 2), (3, 4)],   # v9
    [(0, 3), (4, 4)],   # v16
    [(1, 1), (2, 0)],   # v5
    [(1, 2), (3, 0)],   # v10
    [(1, 3), (4, 0)],   # v17
    [(2, 2), (3, 1)],   # v13
    [(2, 3), (4, 1)],   # v20
    [(3, 3), (4, 2)],   # v25
    [(1, 0)],           # v2
    [(2, 1)],           # v8
    [(3, 2)],           # v18
    [(4, 3)],           # v32
    [(0, 4)],           # v0 center
]
ACL_DX = [[-1, 1], [-2, 2], [-3, 3], [-4, 4], [0]]

# plane-major matmul emission order: each plane's accumulation group is
# consecutive (interleaved start/stop groups mis-accumulate on HW)
_EMIT = []
for _p, _terms in enumerate(PLANE_TERMS):
    for _i, (_td, _ta) in enumerate(sorted(_terms)):
        _EMIT.append((_p, _ta, _td, _i == 0, _i == len(_terms) - 1))
assert len(_EMIT) == 25
# index of the matmul that completes plane 7 (planes 0..7 evacuate first
# for unit 0)
_MID = sum(len(PLANE_TERMS[p]) for p in range(8)) - 1  # = 15

XP_N = C * 264         # 792
G_FLAT = NV * 256      # 3840
BST_FLAT = 5 * 128     # 640
O_FLAT = 2 * C * 128   # 768


def _build_program(nc, bass, mybir):
    bf16 = mybir.dt.bfloat16
    f32 = mybir.dt.float32
    Alu = mybir.AluOpType
    Act = mybir.ActivationFunctionType

    xall_d = nc.declare_dram_parameter("xall", [128, XP_N], bf16, isOutput=False)
    g_d = nc.declare_dram_parameter("g", [128, G_FLAT], bf16, isOutput=False)
    bst_d = nc.declare_dram_parameter("bst", [128, BST_FLAT], bf16, isOutput=False)
    bias_d = nc.declare_dram_parameter("bias", [128, O_FLAT], bf16, isOutput=False)
    out_d = nc.declare_dram_parameter("out", [128, O_FLAT], bf16, isOutput=True)

    xa_sem = nc.alloc_semaphore("xa_sem")      # xp channel 0 (sync queue)
    x1a_sem = nc.alloc_semaphore("x1a_sem")    # xp channels 1,2 (sync queue)
    bst_sem = nc.alloc_semaphore("bst_sem")
    ga_sem = nc.alloc_semaphore("ga_sem")
    gb_sem = nc.alloc_semaphore("gb_sem")
    bis_sem = nc.alloc_semaphore("bis_sem")
    u_sem = nc.alloc_semaphore("u_sem")
    pe_mid_sem = nc.alloc_semaphore("pe_mid_sem")
    pe_sem = nc.alloc_semaphore("pe_sem")
    act_sem = nc.alloc_semaphore("act_sem")
    dh0_sem = nc.alloc_semaphore("dh0_sem")
    dh1_sem = nc.alloc_semaphore("dh1_sem")
    od0_sem = nc.alloc_semaphore("od0_sem")
    od1_sem = nc.alloc_semaphore("od1_sem")

    xall = nc.alloc_sbuf_tensor("s_xall", [128, XP_N], bf16)
    U = nc.alloc_sbuf_tensor("U", [128, 4, C, 256], bf16)
    Ssb = nc.alloc_sbuf_tensor("Ssb", [128, NV, C, 256], bf16)
    # P plane 15 holds the host bias -> 16-term binary tree reduce
    P = nc.alloc_sbuf_tensor("P", [128, 16, C, 256], bf16)
    G = nc.alloc_sbuf_tensor("G", [128, 2, NV, 128], bf16)
    BST = nc.alloc_sbuf_tensor("BST", [128, 5, 128], bf16)
    O = nc.alloc_sbuf_tensor("O", [128, 2, C, 128], bf16)
    scratch = nc.alloc_sbuf_tensor("scratch", [128, 2], bf16)

    ps = [
        nc.alloc_psum_tensor("ps0", [128, NV, 128], f32),
        nc.alloc_psum_tensor("ps1", [128, NV, 128], f32),
    ]

    GA = 1920

    with nc.Block(no_gpsimd_drain=True) as block:
        def flat(t, lo, hi):
            a = t[:]
            return bass.AP(t, lo, [list(a.ap[0]), [1, hi - lo]])

        pd_xall = None  # filled below

        @block.sync
        def _(sync):
            # channel 0 first so the U chain (and PE unit 0) starts early
            sync.dma_start(out=flat(xall, 0, 264), in_=xall_d[:, 0:264]).then_inc(
                xa_sem, 16
            )
            sync.dma_start(out=flat(xall, 264, XP_N), in_=xall_d[:, 264:]).then_inc(
                x1a_sem, 16
            )
            # G half1 (cols 128..255) — not needed until products unit 3
            sync.dma_start(out=flat(G, GA, G_FLAT), in_=g_d[:, GA:]).then_inc(
                ga_sem, 16
            )
            sync.wait_ge(dh0_sem, 1)
            sync.dma_start(out=out_d[:, 0:384], in_=flat(O, 0, 384)).then_inc(
                od0_sem, 16
            )
            sync.wait_ge(dh1_sem, 1)
            sync.dma_start(out=out_d[:, 384:576], in_=flat(O, 384, 576)).then_inc(
                od1_sem, 16
            )
            sync.wait_ge(od0_sem, 16)
            sync.wait_ge(od1_sem, 32)

        @block.gpsimd
        def _(gpsimd):
            gpsimd.wait_ge(od0_sem, 16)
            gpsimd.wait_ge(od1_sem, 32)

        @block.scalar
        def _(scalar):
            # bst first: it gates the PE start and is small; slot-2 spots
            # showed multi-microsecond arrival variance across cores
            scalar.dma_start(out=flat(BST, 0, BST_FLAT), in_=bst_d[:]).then_inc(
                bst_sem, 16
            )
            # G half0 (cols 0..127) — gates the very first products
            scalar.dma_start(out=flat(G, 0, GA), in_=g_d[:, 0:GA]).then_inc(
                gb_sem, 16
            )
            # bias lands directly in P plane 15 (the 16th tree term)
            scalar.dma_start(
                out=flat(P, 15 * C * 256, 16 * C * 256), in_=bias_d[:]
            ).then_inc(bis_sem, 16)
            # dummy op hoists the lazy ACT_TABLE_LOAD off the critical path
            zero_ap = nc.const_aps.aps[(mybir.dt.float32, 0.0)]
            scalar.activation(scratch[:, 0:1], zero_ap, Act.Copy)
            for u in range(6):
                h, c = divmod(u, 3)
                hs = 128 * h
                if u == 0:
                    # split: planes 0..7 as soon as they finish, then 8..14
                    scalar.wait_ge(pe_mid_sem, 1)
                    scalar.activation(
                        Ssb[:, 0:8, c, hs:hs + 128], ps[0][:, 0:8], Act.Copy
                    ).then_inc(act_sem, 1)
                    scalar.wait_ge(pe_sem, 1)
                    scalar.activation(
                        Ssb[:, 8:NV, c, hs:hs + 128], ps[0][:, 8:NV], Act.Copy
                    ).then_inc(act_sem, 1)
                else:
                    scalar.wait_ge(pe_sem, u + 1)
                    scalar.activation(
                        Ssb[:, :, c, hs:hs + 128], ps[u % 2][:], Act.Copy
                    ).then_inc(act_sem, 1)
            scalar.wait_ge(dh1_sem, 1)
            scalar.dma_start(out=out_d[:, 576:768], in_=flat(O, 576, 768)).then_inc(
                od1_sem, 16
            )
            scalar.wait_ge(od1_sem, 32)

        @block.tensor
        def _(tensor):
            pd = list(xall[:].ap[0])
            tensor.wait_ge(bst_sem, 16)
            # dummy matmuls prime the LDWEIGHTS/matmul pipeline while the
            # input DMA finishes (unit 0 otherwise runs ~2x slower);
            # ps[1] is re-zeroed by unit 1's start=True matmuls
            for _ in range(2):
                tensor.matmul(
                    ps[1][:, 0], BST[:, 0], BST[:, 0],
                    start=True, stop=True, skip_group_check=True,
                )
            tensor.wait_ge(xa_sem, 16)
            for u in range(6):
                h, c = divmod(u, 3)
                hs = 128 * h
                tensor.wait_ge(u_sem, c + 1)
                if u >= 2:
                    tensor.wait_ge(act_sem, u)
                for i, (plane, acl, d, st, sp) in enumerate(_EMIT):
                    if acl == 4:
                        rhs = bass.AP(xall, c * 264 + 4 + hs, [pd, [1, 128]])
                    else:
                        rhs = U[:, acl, c, hs:hs + 128]
                    mm = tensor.matmul(
                        ps[u % 2][:, plane],
                        BST[:, d],
                        rhs,
                        start=st,
                        stop=sp,
                        skip_group_check=True,
                    )
                    if u == 0 and i == _MID:
                        mm.then_inc(pe_mid_sem, 1)
                    if i == len(_EMIT) - 1:
                        mm.then_inc(pe_sem, 1)

        @block.vector
        def _(vector):
            pd = list(xall[:].ap[0])
            pdU = list(U[:].ap[0])

            # U column-class sums, per channel: even classes {1,3} hit
            # DVE 2x mode; odd classes {0,2} run at 1x (odd element
            # offsets) — still cheaper than shipping an aligned copy
            vector.wait_ge(xa_sem, 16)
            for c in range(C):
                if c == 1:
                    vector.wait_ge(x1a_sem, 16)
                vector.tensor_tensor(
                    bass.AP(U, 768 + c * 256, [pdU, [1536, 2], [1, 256]]),
                    bass.AP(xall, c * 264 + 2, [pd, [-2, 2], [1, 256]]),
                    bass.AP(xall, c * 264 + 6, [pd, [2, 2], [1, 256]]),
                    Alu.add,
                )
                vector.tensor_tensor(
                    bass.AP(U, c * 256, [pdU, [1536, 2], [1, 256]]),
                    bass.AP(xall, c * 264 + 3, [pd, [-2, 2], [1, 256]]),
                    bass.AP(xall, c * 264 + 5, [pd, [2, 2], [1, 256]]),
                    Alu.add,
                ).then_inc(u_sem, 1)

            def products(lo, hi, c, h):
                hs = 128 * h
                vector.tensor_tensor(
                    P[:, lo:hi, c, hs:hs + 128],
                    Ssb[:, lo:hi, c, hs:hs + 128],
                    G[:, h, lo:hi, :],
                    Alu.mult,
                )

            # 16-term binary tree: 15 products + the bias plane (P[15])
            def tree_l1(h, cs):
                hs = 128 * h
                vector.tensor_tensor(
                    P[:, 0:8, cs, hs:hs + 128],
                    P[:, 0:8, cs, hs:hs + 128],
                    P[:, 8:16, cs, hs:hs + 128],
                    Alu.add,
                )

            def tree_rest(h):
                hs = 128 * h
                vector.tensor_tensor(
                    P[:, 0:4, :, hs:hs + 128],
                    P[:, 0:4, :, hs:hs + 128],
                    P[:, 4:8, :, hs:hs + 128],
                    Alu.add,
                )
                vector.tensor_tensor(
                    P[:, 0:2, :, hs:hs + 128],
                    P[:, 0:2, :, hs:hs + 128],
                    P[:, 2:4, :, hs:hs + 128],
                    Alu.add,
                )
                return vector.tensor_tensor(
                    O[:, h], P[:, 0, :, hs:hs + 128], P[:, 1, :, hs:hs + 128],
                    Alu.add,
                )

            # unit 0 split in two to chase the split evacuation
            vector.wait_ge(gb_sem, 16)
            vector.wait_ge(act_sem, 1)
            products(0, 8, 0, 0)
            vector.wait_ge(act_sem, 2)
            products(8, NV, 0, 0)
            for u in range(1, 6):
                h, c = divmod(u, 3)
                vector.wait_ge(act_sem, u + 2)
                if u == 3:
                    vector.wait_ge(ga_sem, 16)
                products(0, NV, c, h)
                if u == 2:
                    vector.wait_ge(bis_sem, 16)
                    tree_l1(0, slice(0, C))
                    tree_rest(0).then_inc(dh0_sem, 1)
                elif u in (3, 4):
                    # hoist half1's first tree level off the tail, one
                    # channel at a time as its products land
                    tree_l1(1, slice(u - 3, u - 2))
                elif u == 5:
                    tree_l1(1, slice(2, 3))
                    tree_rest(1).then_inc(dh1_sem, 1)

    return nc


_PROGRAM_CACHE = {}


def _get_program():
    if "nc" not in _PROGRAM_CACHE:
        import sys

        if "/opt/trn_rl_repo" not in sys.path:
            sys.path.insert(0, "/opt/trn_rl_repo")
        from concourse import bass, mybir

        nc = bass.Bass()
        _PROGRAM_CACHE["nc"] = _build_program(nc, bass, mybir)
    return _PROGRAM_CACHE["nc"]


def _build_bst():
    bst = np.zeros((5, 128, 128), np.float32)  # [d, in row i, out row r]
    for d in range(5):
        for r in range(128):
            for s in ({d, -d} if d else {0}):
                i = r + s
                if i < 0:
                    i = -i  # top reflect
                if i <= 127:
                    bst[d, i, r] += 1.0
    return bst


def _host_prep(x, foa_xy):
    import ml_dtypes

    bf = ml_dtypes.bfloat16
    x = np.asarray(x)
    bst = _build_bst().transpose(1, 0, 2)  # [i, d, r]
    bst_flat = np.ascontiguousarray(bst.reshape(128, BST_FLAT).astype(bf))
    in_maps = []
    for core in range(N_CORES):
        b, half = divmod(core, 2)
        xb = x[b] if half == 0 else x[b][:, ::-1, :]
        xw = xb[:, 0:132, :]
        xpad = np.pad(xw, ((0, 0), (0, 0), (PAD, PAD)), mode="reflect")  # [3,132,264]
        xp = np.ascontiguousarray(xpad[:, 0:128, :].transpose(1, 0, 2)).astype(bf)
        xall = xp.reshape(128, XP_N)

        rp = np.arange(128)
        yy_img = rp if half == 0 else 255 - rp
        yy, xx = np.meshgrid(
            yy_img.astype(np.float64), np.arange(W, dtype=np.float64), indexing="ij"
        )
        fx, fy = float(foa_xy[b, 0]), float(foa_xy[b, 1])
        dist = np.sqrt((xx - fx) ** 2 + (yy - fy) ** 2)
        dn = dist / DIAG
        sigma = (1.0 - dn) * SIGMA_MIN + dn * SIGMA_MAX
        inv2s2 = 1.0 / (2.0 * sigma * sigma)
        base = -dist * np.sqrt(sigma) / (math.pi * sigma ** 4)
        Gf = np.empty((128, NV, 256), np.float32)
        for i, v in enumerate(V_ORD):
            t = v * inv2s2
            Gf[:, i] = base * (1.0 - t) * np.exp(-t)
        Gf[:, 14] = base

        # bias for out rows 124..127: taps at rows 128..131 (outside window)
        rows = xpad[:, 128:132, :].astype(np.float32)  # [3, 4, 264]
        cs = np.zeros((5, 4, C, 256), np.float32)
        for a in range(5):
            for dx in ACL_DX[a]:
                cs[a] += rows[:, :, 4 + dx:4 + dx + 256].transpose(1, 0, 2)
        bias = np.zeros((128, C, 256), np.float32)
        for plane, terms in enumerate(PLANE_TERMS):
            for (d, acl) in terms:
                if d == 0:
                    continue
                for r in range(124, 128):
                    i = r + d
                    if i >= 128:
                        bias[r] += Gf[r, plane][None, :] * cs[acl, i - 128]
        bias_t = np.ascontiguousarray(bias).astype(bf)  # [128, C, 256]
        # G in column-half-major layout [128, 2, 15, 128]
        g_t = np.ascontiguousarray(
            Gf.astype(bf).reshape(128, NV, 2, 128).transpose(0, 2, 1, 3)
        )

        in_maps.append(
            {
                "xall": np.ascontiguousarray(xall),
                "g": g_t.reshape(128, G_FLAT),
                "bst": bst_flat,
                "bias": bias_t.reshape(128, O_FLAT),
            }
        )
    return in_maps


def _gather(results):
    out = np.empty((B, C, H, W), dtype=np.float32)
    for core in range(N_CORES):
        b, half = divmod(core, 2)
        o = results[core]["out"].astype(np.float32).reshape(128, 2, C, 128)
        o = o.transpose(2, 0, 1, 3).reshape(C, 128, 256)
        if half:
            o = o[:, ::-1, :]
        out[b, :, half * 128:half * 128 + 128, :] = o
    return out


def kernel(x, foa_xy, _trace=False, _tmpdir=None):
    import sys

    if "/opt/trn_rl_repo" not in sys.path:
        sys.path.insert(0, "/opt/trn_rl_repo")
    from concourse.bass_utils import run_bass_kernel_spmd

    nc = _get_program()
    in_maps = _host_prep(np.asarray(x), np.asarray(foa_xy))
    kw = {}
    if _trace:
        kw = dict(trace=True, trace_cores=[], tmpdir=_tmpdir)
    res = run_bass_kernel_spmd(nc, in_maps, list(range(N_CORES)), **kw)
    out = _gather(res.results)
    if _trace:
        return out, res
    return out
